# revision 1
# baseline (speedup 1.0000x reference)
"""Multi-head causal self-attention with RoPE, tensor-parallel over heads
across 8 Trainium2 NeuronCores.

Strategy (Megatron-style TP over heads):
  - Each core owns 2 of the 16 heads: rows [c*256,(c+1)*256) of Wq/Wk/Wv
    and the matching columns of Wo.
  - On-core: qT/kT projections in transposed [d, s] layout (natural matmul
    output layout), RoPE via a signed-permutation matmul + elementwise ops,
    v in natural [s, d] layout, causal attention with scores computed
    transposed (S^T = K Q^T, softmax sum via a ones-matmul, no running max
    needed -- scores are O(10) so exp() cannot overflow), then a partial
    output projection against the core's Wo column-slice.
  - Host sums the 8 partial outputs (this replaces the TP all-reduce).

All matmuls run on float32r operands (full-rate fp32 on the PE). The BIR
verifier requires float32r matmul inputs to be produced as float32r, so
DRAM-sourced operands are declared float32r and engine-produced operands
(RoPE'd q/k, exp(scores), v, u) are written with float32r output dtype.
"""

import sys

import numpy as np

B, S, DIM = 2, 2048, 2048
NUM_HEADS = 16
HD = 128
N_CORES = 8
HPC = NUM_HEADS // N_CORES  # heads per core
DLOC = HPC * HD             # per-core slice of the model dim
ROPE_BASE = 10000.0
QCH = 512                   # attention q-chunk / phase-3 out-chunk
SC1 = 256                   # phase-1 s-chunk

_PROGRAM_CACHE = {}


def _rope_tables_T(seq_len, head_dim):
    # match reference float32 arithmetic: inv_freq over even indices,
    # emb = cat(freqs, freqs); returned transposed [head_dim, seq_len]
    inv_freq = (
        1.0
        / (np.float32(ROPE_BASE)
           ** (np.arange(0, head_dim, 2, dtype=np.float32) / np.float32(head_dim)))
    ).astype(np.float32)
    t = np.arange(seq_len, dtype=np.float32)
    freqs = np.outer(t, inv_freq).astype(np.float32)      # [S, D/2]
    emb = np.concatenate([freqs, freqs], axis=-1)         # [S, D]
    return (
        np.ascontiguousarray(np.cos(emb).astype(np.float32).T),
        np.ascontiguousarray(np.sin(emb).astype(np.float32).T),
    )


def _rot_matrix_T(head_dim):
    # rotated = cat(-x[1::2], x[::2]) = R @ x; return R.T [D, D]
    d2 = head_dim // 2
    R = np.zeros((head_dim, head_dim), dtype=np.float32)
    for dp in range(d2):
        R[dp, 2 * dp + 1] = -1.0
    for dp in range(d2, head_dim):
        R[dp, 2 * (dp - d2)] = 1.0
    return np.ascontiguousarray(R.T)


def _causal_masks(qch):
    # masks[i][kk, qq] = 0 if 128*i + kk <= qq else -1e9 (additive, applied
    # to raw scores before exp, for the 4 diagonal k-chunks of each q-chunk)
    m = np.zeros((4, 128, qch), dtype=np.float32)
    kk = np.arange(128)[:, None]
    qq = np.arange(qch)[None, :]
    for i in range(4):
        m[i] = np.where(128 * i + kk <= qq, 0.0, -1e9).astype(np.float32)
    return m


def build_program(b=B, s=S, dim=DIM):
    """Builds the per-core SPMD Bass program (identical on every core)."""
    if "/opt/trn_rl_repo" not in sys.path:
        sys.path.insert(0, "/opt/trn_rl_repo")
    import concourse.bacc as bacc
    import concourse.mybir as mybir
    import concourse.tile as tile

    f32 = mybir.dt.float32
    f32r = mybir.dt.float32r
    EXP = mybir.ActivationFunctionType.Exp

    bs = b * s
    n_din = dim // 128          # contraction chunks for projections
    n_s1 = bs // SC1            # phase-1 s-chunks
    n_qc = s // QCH             # attention q-chunks per batch
    n_sc3 = bs // 128           # phase-3 row chunks
    n_oc = dim // QCH           # phase-3 out-column chunks
    scale = float(HD) ** -0.5

    nc = bacc.Bacc("TRN2", target_bir_lowering=False, debug=False)

    xT_d = nc.dram_tensor("xT", [dim, bs], f32r, kind="ExternalInput")
    wqT_d = nc.dram_tensor("wqT", [dim, DLOC], f32r, kind="ExternalInput")
    wkT_d = nc.dram_tensor("wkT", [dim, DLOC], f32r, kind="ExternalInput")
    wvT_d = nc.dram_tensor("wvT", [dim, DLOC], f32r, kind="ExternalInput")
    woT_d = nc.dram_tensor("woT", [DLOC, dim], f32r, kind="ExternalInput")
    cosT_d = nc.dram_tensor("cosT", [HD, bs], f32, kind="ExternalInput")
    sinT_d = nc.dram_tensor("sinT", [HD, bs], f32, kind="ExternalInput")
    rT_d = nc.dram_tensor("rT", [HD, HD], f32r, kind="ExternalInput")
    ones_d = nc.dram_tensor("ones", [HD, HD], f32r, kind="ExternalInput")
    masks_d = nc.dram_tensor("masks", [4, HD, QCH], mybir.dt.bfloat16, kind="ExternalInput")
    out_d = nc.dram_tensor("out", [dim, bs], f32, kind="ExternalOutput")

    with tile.TileContext(nc) as tc:
        with tc.tile_pool(name="persist", bufs=1) as persist:
            # transposed roped projections [d, head, b*s]; v natural [s, chunk, d]
            qT = persist.tile([128, HPC, bs], f32r)
            kT = persist.tile([128, HPC, bs], f32r)
            vS = persist.tile([128, bs // 128, DLOC], f32r)
            rTs = persist.tile([HD, HD], f32r)
            nc.sync.dma_start(out=rTs, in_=rT_d[:])
            ones = persist.tile([128, 128], f32r)
            nc.sync.dma_start(out=ones, in_=ones_d[:])
            masks_s = persist.tile([128, 4, QCH], mybir.dt.bfloat16)
            nc.sync.dma_start(out=masks_s, in_=masks_d.rearrange("i p q -> p i q"))
            woT_s = persist.tile([128, HPC, dim], f32r)
            nc.sync.dma_start(
                out=woT_s, in_=woT_d.rearrange("(h p) n -> p h n", p=128)
            )

            # ---------------- phase 1: qkv projections + RoPE ----------------
            with (
                tc.tile_pool(name="p1w", bufs=1) as p1w,
                tc.tile_pool(name="p1x", bufs=2) as p1x,
                tc.tile_pool(name="p1t", bufs=2) as p1t,
                tc.tile_pool(name="ps_qk", bufs=4, space="PSUM") as ps_qk,
                tc.tile_pool(name="ps_rot", bufs=2, space="PSUM") as ps_rot,
                tc.tile_pool(name="ps_v", bufs=2, space="PSUM") as ps_v,
            ):
                wq_s = p1w.tile([128, n_din, DLOC], f32r)
                wk_s = p1w.tile([128, n_din, DLOC], f32r)
                wv_s = p1w.tile([128, n_din, DLOC], f32r)
                # split weight loads so the first matmuls start as soon as the
                # first contraction chunks land (DMA queues run in parallel)
                gw = max(1, n_din // 4)
                for g0 in range(0, n_din, gw):
                    for w_t, w_d in ((wq_s, wqT_d), (wk_s, wkT_d), (wv_s, wvT_d)):
                        nc.sync.dma_start(
                            out=w_t[:, g0 : g0 + gw, :],
                            in_=w_d.rearrange("(c p) m -> p c m", p=128)[
                                :, g0 : g0 + gw, :
                            ],
                        )

                for si in range(n_s1):
                    s0 = si * SC1
                    xt = p1x.tile([128, n_din, SC1], f32r, tag="xt")
                    xsrc = xT_d[:, s0 : s0 + SC1].rearrange("(c p) s -> p c s", p=128)
                    nh = n_din // 2
                    nc.sync.dma_start(out=xt[:, :nh, :], in_=xsrc[:, :nh, :])
                    nc.sync.dma_start(out=xt[:, nh:, :], in_=xsrc[:, nh:, :])
                    cost = p1x.tile([128, SC1], f32, tag="cost")
                    nc.sync.dma_start(out=cost, in_=cosT_d[:, s0 : s0 + SC1])
                    sint = p1x.tile([128, SC1], f32, tag="sint")
                    nc.sync.dma_start(out=sint, in_=sinT_d[:, s0 : s0 + SC1])

                    for w_s, store in ((wq_s, qT), (wk_s, kT)):
                        for h in range(HPC):
                            acc = ps_qk.tile([128, SC1], f32, tag="qk")
                            for c in range(n_din):
                                nc.tensor.matmul(
                                    acc,
                                    lhsT=w_s[:, c, h * HD : (h + 1) * HD],
                                    rhs=xt[:, c, :],
                                    start=(c == 0),
                                    stop=(c == n_din - 1),
                                )
                            raw = p1t.tile([128, SC1], f32r, tag="raw")
                            nc.scalar.copy(raw, acc)
                            rot = ps_rot.tile([128, SC1], f32, tag="rot")
                            nc.tensor.matmul(
                                rot, lhsT=rTs, rhs=raw, start=True, stop=True
                            )
                            t1 = p1t.tile([128, SC1], f32, tag="t1")
                            nc.vector.tensor_mul(t1, raw.bitcast(f32), cost)
                            t2 = p1t.tile([128, SC1], f32, tag="t2")
                            nc.vector.tensor_mul(t2, rot, sint)
                            nc.vector.tensor_add(store[:, h, s0 : s0 + SC1], t1, t2)

                    for sub in range(SC1 // 128):
                        vacc = ps_v.tile([128, DLOC], f32, tag="v")
                        for c in range(n_din):
                            nc.tensor.matmul(
                                vacc,
                                lhsT=xt[:, c, sub * 128 : (sub + 1) * 128],
                                rhs=wv_s[:, c, :],
                                start=(c == 0),
                                stop=(c == n_din - 1),
                            )
                        nc.scalar.copy(vS[:, s0 // 128 + sub, :], vacc)

            # ------------- phases 2+3: attention, then output projection -------------
            # pools for both phases coexist so phase-3 groups (per batch) can
            # start while later batches' attention is still running
            with (
                tc.tile_pool(name="persistB", bufs=1) as persistB,
                tc.tile_pool(name="p2", bufs=4) as p2,
                tc.tile_pool(name="p2l", bufs=3) as p2l,
                tc.tile_pool(name="p2r", bufs=2) as p2r,
                tc.tile_pool(name="p3", bufs=2) as p3,
                tc.tile_pool(name="ps_st", bufs=2, space="PSUM") as ps_st,
                tc.tile_pool(name="ps_o", bufs=2, space="PSUM") as ps_o,
                tc.tile_pool(name="ps3", bufs=4, space="PSUM") as ps3,
            ):
                uT = persistB.tile([128, HPC, bs], f32r)  # attn out, [d, h, b*s]

                SCG = min(2, s // QCH)
                n_scg_b = s // (SCG * QCH)  # phase-3 groups per batch

                def phase3_groups(bi):
                    # outT[dout, s] = woT.T @ uT for batch bi's s-range;
                    # emitted right after bi's attention so the PE queue
                    # pipelines projection bursts with attention tails
                    for oc in range(dim // 128):
                        o0 = oc * 128
                        for gl in range(n_scg_b):
                            g = bi * n_scg_b + gl
                            pos = [
                                ps3.tile([128, QCH], f32, tag="op", name=f"po{_j}")
                                for _j in range(SCG)
                            ]
                            for h in range(HPC):
                                for j in range(SCG):
                                    s0 = (g * SCG + j) * QCH
                                    nc.tensor.matmul(
                                        pos[j],
                                        lhsT=woT_s[:, h, o0 : o0 + 128],
                                        rhs=uT[:, h, s0 : s0 + QCH],
                                        start=(h == 0),
                                        stop=(h == HPC - 1),
                                    )
                            ot = p3.tile([128, SCG, QCH], f32, tag="ot")
                            for j in range(SCG):
                                if j % 2 == 0:
                                    nc.scalar.copy(ot[:, j, :], pos[j])
                                else:
                                    nc.vector.tensor_copy(ot[:, j, :], pos[j])
                            nc.sync.dma_start(
                                out=out_d[
                                    o0 : o0 + 128, g * SCG * QCH : (g + 1) * SCG * QCH
                                ],
                                in_=ot,
                            )

                for bi in range(b):
                    for h in range(HPC):
                        for qc in range(n_qc):
                            q0 = bi * s + qc * QCH
                            nkc = (qc + 1) * QCH // 128
                            outp = ps_o.tile([128, QCH], f32, tag="o")
                            lrep = ps_o.tile([128, QCH], f32, tag="o", name="lrep")
                            prev_pt = None
                            li = 0
                            for kc in range(nkc):
                                k0 = bi * s + kc * 128
                                st = ps_st.tile([128, QCH], f32, tag="st")
                                nc.tensor.matmul(
                                    st,
                                    lhsT=kT[:, h, k0 : k0 + 128],
                                    rhs=qT[:, h, q0 : q0 + QCH],
                                    start=True,
                                    stop=True,
                                )
                                di = kc - (nkc - 4)
                                if di >= 0:
                                    # additive -1e9 causal mask on raw scores
                                    nc.vector.tensor_add(st, st, masks_s[:, di, :])
                                pt = p2.tile([128, QCH], f32r, tag="pt")
                                nc.scalar.activation(pt, st, EXP, scale=scale)
                                nc.tensor.matmul(
                                    outp,
                                    lhsT=vS[
                                        :, bi * (s // 128) + kc, h * HD : (h + 1) * HD
                                    ],
                                    rhs=pt,
                                    start=(kc == 0),
                                    stop=(kc == nkc - 1),
                                )
                                if kc % 2 == 1:
                                    # softmax denominator: independent pair-sums
                                    # (DVE/GpSimd alternating), partition-reduced
                                    # by an interleaved ones-matmul accumulation
                                    lp = p2l.tile([128, QCH], f32r, tag="lp")
                                    eng = nc.vector if li % 2 == 0 else nc.gpsimd
                                    eng.tensor_add(lp, prev_pt, pt)
                                    nc.tensor.matmul(
                                        lrep,
                                        lhsT=ones,
                                        rhs=lp,
                                        start=(li == 0),
                                        stop=(li == nkc // 2 - 1),
                                    )
                                    li += 1
                                prev_pt = pt
                            rec = p2r.tile([128, QCH], f32, tag="rec")
                            nc.vector.reciprocal_approx_fast(rec, lrep)
                            nc.vector.tensor_mul(uT[:, h, q0 : q0 + QCH], outp, rec)
                    phase3_groups(bi)

    nc.compile()
    return nc


def make_in_maps(x, Wq, Wk, Wv, Wo, b=B, s=S, dim=DIM, n_cores=N_CORES):
    bs = b * s
    xT = np.ascontiguousarray(x.reshape(bs, dim).T.astype(np.float32))
    cosT1, sinT1 = _rope_tables_T(s, HD)
    cosT = np.ascontiguousarray(np.tile(cosT1, (1, b)))
    sinT = np.ascontiguousarray(np.tile(sinT1, (1, b)))
    rT = _rot_matrix_T(HD)
    ones = np.ones((HD, HD), dtype=np.float32)
    import ml_dtypes
    masks = _causal_masks(QCH).astype(ml_dtypes.bfloat16)
    in_maps = []
    for c in range(n_cores):
        sl = slice(c * DLOC, (c + 1) * DLOC)
        in_maps.append(
            {
                "xT": xT,
                "wqT": np.ascontiguousarray(Wq[sl, :].T.astype(np.float32)),
                "wkT": np.ascontiguousarray(Wk[sl, :].T.astype(np.float32)),
                "wvT": np.ascontiguousarray(Wv[sl, :].T.astype(np.float32)),
                "woT": np.ascontiguousarray(Wo[:, sl].T.astype(np.float32)),
                "cosT": cosT,
                "sinT": sinT,
                "rT": rT,
                "ones": ones,
                "masks": masks,
            }
        )
    return in_maps


def kernel(x, Wq, Wk, Wv, Wo, _trace=False):
    """Full-input / full-output entry point. Shards over 8 cores internally."""
    if "/opt/trn_rl_repo" not in sys.path:
        sys.path.insert(0, "/opt/trn_rl_repo")
    from concourse.bass_utils import run_bass_kernel_spmd

    x = np.asarray(x, dtype=np.float32)
    Wq, Wk, Wv, Wo = (np.asarray(w, dtype=np.float32) for w in (Wq, Wk, Wv, Wo))

    key = (B, S, DIM)
    if key not in _PROGRAM_CACHE:
        _PROGRAM_CACHE[key] = build_program(B, S, DIM)
    nc = _PROGRAM_CACHE[key]

    in_maps = make_in_maps(x, Wq, Wk, Wv, Wo)
    res = run_bass_kernel_spmd(
        nc, in_maps, core_ids=list(range(N_CORES)), trace=_trace
    )
    kernel.last_results = res
    acc = res.results[0]["out"].astype(np.float32)
    for c in range(1, N_CORES):
        acc = acc + res.results[c]["out"]
    return np.ascontiguousarray(acc.T).reshape(B, S, DIM)



# revision 2
# speedup vs baseline: 1.3226x; 1.3226x over previous
"""Multi-head causal self-attention with RoPE on 8 Trainium2 NeuronCores.

Sharding: DP(2) x TP(4). Cores [4g, 4g+4) own batch g; within a group,
core r owns heads [4r, 4r+4) (rows [r*512,(r+1)*512) of Wq/Wk/Wv and the
matching columns of Wo). The host sums the 4 partial output projections
per batch (replaces the TP all-reduce) -- partial sums travel as fp16.

On-core layout (all matmul operands fp16, PSUM accumulation fp32):
  - qT/kT projections in transposed [d, s] layout; RoPE via a signed
    permutation matmul + elementwise ops (software-pipelined by one chain
    so the rot matmul never stalls the PE FIFO).
  - v in natural [s, d] layout.
  - Scores computed transposed (S^T = K Q^T); no running max needed
    (scores*scale are O(5), exp cannot overflow fp16 range).
  - Causal handling: k-chunks above the diagonal band are skipped, the
    diagonal band is column-trimmed (N = 512-128*di) and only the
    [128,128] triangular block is masked, multiplicatively after exp.
  - Softmax denominator via a ones-matmul accumulated per k-chunk; the
    attention inner loop is software-pipelined (scores run one chunk
    ahead of AV/ones) so exp latency stays off the PE critical path.
  - Output projection interleaved per q-chunk, one head-chain late.
"""

import sys

import numpy as np

B, S, DIM = 2, 2048, 2048
NUM_HEADS = 16
HD = 128
N_CORES = 8
DP = 2                       # data-parallel groups (one batch each)
TP = N_CORES // DP           # tensor-parallel ranks per group
HPC = NUM_HEADS // TP        # heads per core (4)
DLOC = HPC * HD              # per-core slice of the model dim (512)
ROPE_BASE = 10000.0
SC = 512                     # s-chunk for projections / attention q-chunk

_PROGRAM_CACHE = {}


def _rope_tables_T(seq_len, head_dim):
    # match reference float32 arithmetic: inv_freq over even indices,
    # emb = cat(freqs, freqs); returned transposed [head_dim, seq_len]
    inv_freq = (
        1.0
        / (np.float32(ROPE_BASE)
           ** (np.arange(0, head_dim, 2, dtype=np.float32) / np.float32(head_dim)))
    ).astype(np.float32)
    t = np.arange(seq_len, dtype=np.float32)
    freqs = np.outer(t, inv_freq).astype(np.float32)      # [S, D/2]
    emb = np.concatenate([freqs, freqs], axis=-1)         # [S, D]
    return (
        np.ascontiguousarray(np.cos(emb).astype(np.float16).T),
        np.ascontiguousarray(np.sin(emb).astype(np.float16).T),
    )


def _rot_matrix_T(head_dim):
    # rotated = cat(-x[1::2], x[::2]) = R @ x; return R.T [D, D]
    d2 = head_dim // 2
    R = np.zeros((head_dim, head_dim), dtype=np.float16)
    for dp in range(d2):
        R[dp, 2 * dp + 1] = -1.0
    for dp in range(d2, head_dim):
        R[dp, 2 * (dp - d2)] = 1.0
    return np.ascontiguousarray(R.T)


def _tri01():
    # tri01[kk, qq] = 1 if kk <= qq else 0 (multiplicative causal mask for
    # the [128,128] diagonal block of every diagonal k-chunk)
    kk = np.arange(128)[:, None]
    qq = np.arange(128)[None, :]
    return np.ascontiguousarray((kk <= qq).astype(np.float16))


def build_program(s=S, dim=DIM):
    """Per-core SPMD Bass program (identical on every core)."""
    if "/opt/trn_rl_repo" not in sys.path:
        sys.path.insert(0, "/opt/trn_rl_repo")
    import concourse.bacc as bacc
    import concourse.mybir as mybir
    import concourse.tile as tile

    f32 = mybir.dt.float32
    f16 = mybir.dt.float16
    EXP = mybir.ActivationFunctionType.Exp

    n_din = dim // 128          # contraction chunks for projections (16)
    n_sc = s // SC              # s-chunks (4)
    scale = float(HD) ** -0.5

    nc = bacc.Bacc("TRN2", target_bir_lowering=False, debug=False)

    xT_d = nc.dram_tensor("xT", [dim, s], f16, kind="ExternalInput")
    wqT_d = nc.dram_tensor("wqT", [dim, DLOC], f16, kind="ExternalInput")
    wkT_d = nc.dram_tensor("wkT", [dim, DLOC], f16, kind="ExternalInput")
    wvT_d = nc.dram_tensor("wvT", [dim, DLOC], f16, kind="ExternalInput")
    woT_d = nc.dram_tensor("woT", [DLOC, dim], f16, kind="ExternalInput")
    cosT_d = nc.dram_tensor("cosT", [HD, s], f16, kind="ExternalInput")
    sinT_d = nc.dram_tensor("sinT", [HD, s], f16, kind="ExternalInput")
    rT_d = nc.dram_tensor("rT", [HD, HD], f16, kind="ExternalInput")
    ones_d = nc.dram_tensor("ones", [HD, HD], f16, kind="ExternalInput")
    tri_d = nc.dram_tensor("tri", [HD, HD], f16, kind="ExternalInput")
    out_d = nc.dram_tensor("out", [dim, s], f16, kind="ExternalOutput")

    with tile.TileContext(nc) as tc:
        with tc.tile_pool(name="persist", bufs=1) as persist:
            qT = persist.tile([128, HPC, s], f16)   # roped q, [d, h, s]
            kT = persist.tile([128, HPC, s], f16)
            vS = persist.tile([128, s // 128, DLOC], f16)  # [k, chunk, d]
            uT = persist.tile([128, HPC, s], f16)   # attention out, [d, h, s]
            rTs = persist.tile([HD, HD], f16)
            nc.sync.dma_start(out=rTs, in_=rT_d[:])
            ones = persist.tile([HD, HD], f16)
            nc.sync.dma_start(out=ones, in_=ones_d[:])
            tri01 = persist.tile([HD, HD], f16)
            nc.sync.dma_start(out=tri01, in_=tri_d[:])
            cosT = persist.tile([HD, s], f16)
            nc.sync.dma_start(out=cosT, in_=cosT_d[:])
            sinT = persist.tile([HD, s], f16)
            nc.sync.dma_start(out=sinT, in_=sinT_d[:])
            woT_s = persist.tile([128, HPC, dim], f16)
            nc.sync.dma_start(
                out=woT_s, in_=woT_d.rearrange("(h p) n -> p h n", p=128)
            )

            # ---------------- phase 1: qkv projections + RoPE ----------------
            with (
                tc.tile_pool(name="p1w", bufs=1) as p1w,
                tc.tile_pool(name="p1x", bufs=2) as p1x,
                tc.tile_pool(name="p1t", bufs=2) as p1t,
                tc.tile_pool(name="ps1", bufs=2, space="PSUM") as ps1,
            ):
                wq_s = p1w.tile([128, n_din, DLOC], f16)
                wk_s = p1w.tile([128, n_din, DLOC], f16)
                wv_s = p1w.tile([128, n_din, DLOC], f16)
                # split weight loads so the first matmuls start as soon as the
                # first contraction chunks land (DMA queues run in parallel)
                gw = n_din // 4
                for g0 in range(0, n_din, gw):
                    for w_t, w_d in ((wq_s, wqT_d), (wk_s, wkT_d), (wv_s, wvT_d)):
                        nc.sync.dma_start(
                            out=w_t[:, g0 : g0 + gw, :],
                            in_=w_d.rearrange("(c p) m -> p c m", p=128)[
                                :, g0 : g0 + gw, :
                            ],
                        )

                def finish_rope(raw, store, h, s0):
                    # rot matmul emitted one chain late so the PE never waits
                    # on the scalar-engine raw copy
                    rot = ps1.tile([128, SC], f32, tag="rot")
                    nc.tensor.matmul(rot, lhsT=rTs, rhs=raw, start=True, stop=True)
                    t1 = p1t.tile([128, SC], f16, tag="t1")
                    nc.vector.tensor_mul(t1, raw, cosT[:, s0 : s0 + SC])
                    t2 = p1t.tile([128, SC], f16, tag="t2")
                    nc.vector.tensor_mul(t2, rot, sinT[:, s0 : s0 + SC])
                    nc.gpsimd.tensor_add(store[:, h, s0 : s0 + SC], t1, t2)

                pending = None
                for si in range(n_sc):
                    s0 = si * SC
                    xt = p1x.tile([128, n_din, SC], f16, tag="xt")
                    xsrc = xT_d[:, s0 : s0 + SC].rearrange("(c p) s -> p c s", p=128)
                    gx = n_din // 4
                    for g0 in range(0, n_din, gx):
                        nc.sync.dma_start(
                            out=xt[:, g0 : g0 + gx, :], in_=xsrc[:, g0 : g0 + gx, :]
                        )

                    for w_s, store in ((wq_s, qT), (wk_s, kT)):
                        for h in range(HPC):
                            acc = ps1.tile([128, SC], f32, tag="acc")
                            for c in range(n_din):
                                nc.tensor.matmul(
                                    acc,
                                    lhsT=w_s[:, c, h * HD : (h + 1) * HD],
                                    rhs=xt[:, c, :],
                                    start=(c == 0),
                                    stop=(c == n_din - 1),
                                )
                            raw = p1t.tile([128, SC], f16, tag="raw", bufs=3)
                            nc.scalar.copy(raw, acc)
                            if pending is not None:
                                finish_rope(*pending)
                            pending = (raw, store, h, s0)

                    for sub in range(SC // 128):
                        vacc = ps1.tile([128, DLOC], f32, tag="vacc")
                        for c in range(n_din):
                            nc.tensor.matmul(
                                vacc,
                                lhsT=xt[:, c, sub * 128 : (sub + 1) * 128],
                                rhs=wv_s[:, c, :],
                                start=(c == 0),
                                stop=(c == n_din - 1),
                            )
                        if pending is not None:
                            finish_rope(*pending)
                            pending = None
                        vdst = vS[:, si * (SC // 128) + sub, :]
                        if sub % 2 == 0:
                            nc.scalar.copy(vdst, vacc)
                        else:
                            nc.vector.tensor_copy(vdst, vacc)

            # ------------- phase 2+3: attention + output projection -------------
            with (
                tc.tile_pool(name="p2", bufs=4) as p2,
                tc.tile_pool(name="p2r", bufs=2) as p2r,
                tc.tile_pool(name="p3", bufs=3) as p3,
                tc.tile_pool(name="ps_t", bufs=3, space="PSUM") as ps_t,
                tc.tile_pool(name="ps_o", bufs=2, space="PSUM") as ps_o,
                tc.tile_pool(name="ps_l", bufs=2, space="PSUM") as ps_l,
            ):
                def phase3(qc):
                    # outT[n, s] = woT.T @ uT for q-chunk qc (all heads)
                    q0 = qc * SC
                    for oc in range(dim // 128):
                        o0 = oc * 128
                        pos = ps_t.tile([128, SC], f32, tag="st", name="pos")
                        for h in range(HPC):
                            nc.tensor.matmul(
                                pos,
                                lhsT=woT_s[:, h, o0 : o0 + 128],
                                rhs=uT[:, h, q0 : q0 + SC],
                                start=(h == 0),
                                stop=(h == HPC - 1),
                            )
                        ot = p3.tile([128, SC], f16, tag="ot")
                        if oc % 2 == 0:
                            nc.scalar.copy(ot, pos)
                        else:
                            nc.vector.tensor_copy(ot, pos)
                        nc.sync.dma_start(
                            out=out_d[o0 : o0 + 128, q0 : q0 + SC], in_=ot
                        )

                for qc in range(n_sc):
                    q0 = qc * SC
                    nkc = (qc + 1) * (SC // 128)
                    for h in range(HPC):
                        outp = ps_o.tile([128, SC], f32, tag="o")
                        lrep = ps_l.tile([128, SC], f32, tag="l")
                        # software pipeline: scores/exp run one chunk ahead
                        # of the AV/ones accumulation so exp latency hides
                        pend = None
                        for kc in range(nkc):
                            di = kc - (SC // 128) * qc
                            co = 128 * di if di > 0 else 0
                            st = ps_t.tile([128, SC], f32, tag="st")
                            nc.tensor.matmul(
                                st[:, co:],
                                lhsT=kT[:, h, kc * 128 : (kc + 1) * 128],
                                rhs=qT[:, h, q0 + co : q0 + SC],
                                start=True,
                                stop=True,
                            )
                            pt = p2.tile([128, SC], f16, tag="pt")
                            nc.scalar.activation(pt[:, co:], st[:, co:], EXP, scale=scale)
                            if di >= 0:
                                nc.vector.tensor_mul(
                                    pt[:, co : co + 128], pt[:, co : co + 128], tri01
                                )
                            if pend is not None:
                                pco, ppt, pkc = pend
                                nc.tensor.matmul(
                                    outp[:, pco:],
                                    lhsT=vS[:, pkc, h * HD : (h + 1) * HD],
                                    rhs=ppt[:, pco:],
                                    start=(pkc == 0),
                                    stop=False,
                                )
                                nc.tensor.matmul(
                                    lrep[:, pco:],
                                    lhsT=ones,
                                    rhs=ppt[:, pco:],
                                    start=(pkc == 0),
                                    stop=False,
                                )
                            pend = (co, pt, kc)
                        pco, ppt, pkc = pend
                        nc.tensor.matmul(
                            outp[:, pco:],
                            lhsT=vS[:, pkc, h * HD : (h + 1) * HD],
                            rhs=ppt[:, pco:],
                            start=(pkc == 0),
                            stop=True,
                        )
                        nc.tensor.matmul(
                            lrep[:, pco:],
                            lhsT=ones,
                            rhs=ppt[:, pco:],
                            start=(pkc == 0),
                            stop=True,
                        )
                        rec = p2r.tile([128, SC], f32, tag="rec")
                        nc.vector.reciprocal_approx_fast(rec, lrep)
                        nc.vector.tensor_mul(uT[:, h, q0 : q0 + SC], outp, rec)
                        if h == 0 and qc > 0:
                            phase3(qc - 1)
                phase3(n_sc - 1)

    nc.compile()
    return nc


def make_in_maps(x, Wq, Wk, Wv, Wo):
    cosT, sinT = _rope_tables_T(S, HD)
    rT = _rot_matrix_T(HD)
    ones = np.ones((HD, HD), dtype=np.float16)
    tri = _tri01()
    xT = [
        np.ascontiguousarray(x[g].T.astype(np.float16)) for g in range(DP)
    ]
    in_maps = []
    for c in range(N_CORES):
        g, r = divmod(c, TP)
        sl = slice(r * DLOC, (r + 1) * DLOC)
        in_maps.append(
            {
                "xT": xT[g],
                "wqT": np.ascontiguousarray(Wq[sl, :].T.astype(np.float16)),
                "wkT": np.ascontiguousarray(Wk[sl, :].T.astype(np.float16)),
                "wvT": np.ascontiguousarray(Wv[sl, :].T.astype(np.float16)),
                "woT": np.ascontiguousarray(Wo[:, sl].T.astype(np.float16)),
                "cosT": cosT,
                "sinT": sinT,
                "rT": rT,
                "ones": ones,
                "tri": tri,
            }
        )
    return in_maps


def kernel(x, Wq, Wk, Wv, Wo, _trace=False):
    """Full-input / full-output entry point. Shards over 8 cores internally."""
    if "/opt/trn_rl_repo" not in sys.path:
        sys.path.insert(0, "/opt/trn_rl_repo")
    from concourse.bass_utils import run_bass_kernel_spmd

    x = np.asarray(x, dtype=np.float32)
    Wq, Wk, Wv, Wo = (np.asarray(w, dtype=np.float32) for w in (Wq, Wk, Wv, Wo))

    key = (B, S, DIM)
    if key not in _PROGRAM_CACHE:
        _PROGRAM_CACHE[key] = build_program(S, DIM)
    nc = _PROGRAM_CACHE[key]

    in_maps = make_in_maps(x, Wq, Wk, Wv, Wo)
    res = run_bass_kernel_spmd(
        nc, in_maps, core_ids=list(range(N_CORES)), trace=_trace
    )
    kernel.last_results = res
    out = np.empty((B, S, DIM), dtype=np.float32)
    for g in range(DP):
        acc = res.results[g * TP]["out"].astype(np.float32)
        for r in range(1, TP):
            acc = acc + res.results[g * TP + r]["out"].astype(np.float32)
        out[g] = acc.T
    return out


# revision 4
# speedup vs baseline: 1.3994x; 1.0581x over previous
"""Multi-head causal self-attention with RoPE on 8 Trainium2 NeuronCores.

Sharding: DP(2) x TP(4). Cores [4g, 4g+4) own batch g; within a group,
core r owns heads [4r, 4r+4) (rows [r*512,(r+1)*512) of Wq/Wk/Wv and the
matching columns of Wo). The host sums the 4 partial output projections
per batch (replaces the TP all-reduce); partial sums travel as fp16.

Performance notes (measured on TRN2):
  - PE matmul issue rate is N cycles @2.4GHz regardless of operand dtype
    (fp16 == bf16 == weight-reuse); the kernel is PE-streaming-bound, so
    everything else is organized to keep the PE FIFO dense.
  - dma_start issue on the Sync engine costs ~2.7ns per descriptor line;
    all DRAM tensors are pre-tiled on the host so every transfer is 128
    long per-partition-contiguous descriptors (~0.35us issue each).
  - Scalar activations pay a ~370-cycle access-latency adder, so exps are
    batched two k-chunks per call (st pairs span 2 PSUM banks; engines
    other than the PE may read across banks).
  - Softmax denominators: full (sub-diagonal) k-chunks are summed in
    quads on the DVE (fp16, 2x mode) with one ones-matmul per quad;
    diagonal chunks get individual column-trimmed ones-matmuls.
  - Causality: diagonal-band score chunks are column-trimmed to
    N = 512-128*di and masked multiplicatively (after exp) with a single
    [128,128] triangular 0/1 mask.
"""

import sys

import numpy as np

B, S, DIM = 2, 2048, 2048
NUM_HEADS = 16
HD = 128
N_CORES = 8
DP = 2                       # data-parallel groups (one batch each)
TP = N_CORES // DP           # tensor-parallel ranks per group
HPC = NUM_HEADS // TP        # heads per core (4)
DLOC = HPC * HD              # per-core slice of the model dim (512)
ROPE_BASE = 10000.0
SC = 512                     # s-chunk for projections / attention q-chunk

_PROGRAM_CACHE = {}


def _rope_tables_T(seq_len, head_dim):
    # match reference float32 arithmetic: inv_freq over even indices,
    # emb = cat(freqs, freqs); returned transposed [head_dim, seq_len]
    inv_freq = (
        1.0
        / (np.float32(ROPE_BASE)
           ** (np.arange(0, head_dim, 2, dtype=np.float32) / np.float32(head_dim)))
    ).astype(np.float32)
    t = np.arange(seq_len, dtype=np.float32)
    freqs = np.outer(t, inv_freq).astype(np.float32)      # [S, D/2]
    emb = np.concatenate([freqs, freqs], axis=-1)         # [S, D]
    return (
        np.ascontiguousarray(np.cos(emb).astype(np.float16).T),
        np.ascontiguousarray(np.sin(emb).astype(np.float16).T),
    )


def _rot_matrix_T(head_dim):
    # rotated = cat(-x[1::2], x[::2]) = R @ x; return R.T [D, D]
    d2 = head_dim // 2
    R = np.zeros((head_dim, head_dim), dtype=np.float16)
    for dp in range(d2):
        R[dp, 2 * dp + 1] = -1.0
    for dp in range(d2, head_dim):
        R[dp, 2 * (dp - d2)] = 1.0
    return np.ascontiguousarray(R.T)


def _tri01():
    # tri01[kk, qq] = 1 if kk <= qq else 0 (multiplicative causal mask for
    # the [128,128] diagonal block of every diagonal k-chunk)
    kk = np.arange(128)[:, None]
    qq = np.arange(128)[None, :]
    return np.ascontiguousarray((kk <= qq).astype(np.float16))


def build_program(s=S, dim=DIM):
    """Per-core SPMD Bass program (identical on every core)."""
    if "/opt/trn_rl_repo" not in sys.path:
        sys.path.insert(0, "/opt/trn_rl_repo")
    import concourse.bacc as bacc
    import concourse.mybir as mybir
    import concourse.tile as tile

    f32 = mybir.dt.float32
    f16 = mybir.dt.float16
    EXP = mybir.ActivationFunctionType.Exp

    n_din = dim // 128          # contraction chunks for projections (16)
    n_sc = s // SC              # s-chunks (4)
    n_oc = dim // 128           # output-projection row chunks (16)
    scale = float(HD) ** -0.5

    nc = bacc.Bacc("TRN2", target_bir_lowering=False, debug=False)

    # all DRAM tensors pre-tiled on the host: partition dim first, then
    # per-partition-contiguous free dims, so DMAs are 128 fat descriptors
    x_d = nc.dram_tensor("x", [128, n_sc, n_din, SC], f16, kind="ExternalInput")
    wq_d = nc.dram_tensor("wq", [128, n_din, DLOC], f16, kind="ExternalInput")
    wk_d = nc.dram_tensor("wk", [128, n_din, DLOC], f16, kind="ExternalInput")
    wv_d = nc.dram_tensor("wv", [128, n_din, DLOC], f16, kind="ExternalInput")
    wo_d = nc.dram_tensor("wo", [128, HPC, dim], f16, kind="ExternalInput")
    cosT_d = nc.dram_tensor("cosT", [HD, s], f16, kind="ExternalInput")
    sinT_d = nc.dram_tensor("sinT", [HD, s], f16, kind="ExternalInput")
    rT_d = nc.dram_tensor("rT", [HD, HD], f16, kind="ExternalInput")
    ones_d = nc.dram_tensor("ones", [HD, HD], f16, kind="ExternalInput")
    tri_d = nc.dram_tensor("tri", [HD, HD], f16, kind="ExternalInput")
    out_d = nc.dram_tensor("out", [128, n_sc, n_oc, SC], f16, kind="ExternalOutput")

    with tile.TileContext(nc) as tc:
        with tc.tile_pool(name="persist", bufs=1) as persist:
            qT = persist.tile([128, HPC, s], f16)   # roped q, [d, h, s]
            kT = persist.tile([128, HPC, s], f16)
            vS = persist.tile([128, s // 128, DLOC], f16)  # [k, chunk, d]
            uT = persist.tile([128, HPC, s], f16)   # attention out, [d, h, s]

            # ---------------- phase 1: qkv projections + RoPE ----------------
            with (
                tc.tile_pool(name="p1x", bufs=3) as p1x,
                tc.tile_pool(name="p1w", bufs=1) as p1w,
                tc.tile_pool(name="p1t", bufs=2) as p1t,
                tc.tile_pool(name="ps1", bufs=2, space="PSUM") as ps1,
            ):
                # first x chunk + q weights go out first so the PE starts early
                xts = []
                for si in range(3):
                    xt = p1x.tile([128, n_din, SC], f16, tag="xt", name=f"xt{si}")
                    nc.sync.dma_start(out=xt, in_=x_d[:, si, :, :])
                    xts.append(xt)
                wq_s = p1w.tile([128, n_din, DLOC], f16)
                nh = n_din // 2
                nc.sync.dma_start(out=wq_s[:, :nh, :], in_=wq_d[:, :nh, :])
                nc.sync.dma_start(out=wq_s[:, nh:, :], in_=wq_d[:, nh:, :])
                rTs = persist.tile([HD, HD], f16)
                nc.sync.dma_start(out=rTs, in_=rT_d[:])
                cosT = persist.tile([HD, s], f16)
                nc.sync.dma_start(out=cosT, in_=cosT_d[:])
                sinT = persist.tile([HD, s], f16)
                nc.sync.dma_start(out=sinT, in_=sinT_d[:])
                wk_s = p1w.tile([128, n_din, DLOC], f16)
                nc.sync.dma_start(out=wk_s[:, :nh, :], in_=wk_d[:, :nh, :])
                nc.sync.dma_start(out=wk_s[:, nh:, :], in_=wk_d[:, nh:, :])
                wv_s = p1w.tile([128, n_din, DLOC], f16)
                nc.sync.dma_start(out=wv_s[:, :nh, :], in_=wv_d[:, :nh, :])
                nc.sync.dma_start(out=wv_s[:, nh:, :], in_=wv_d[:, nh:, :])
                ones = persist.tile([HD, HD], f16)
                nc.sync.dma_start(out=ones, in_=ones_d[:])
                tri01 = persist.tile([HD, HD], f16)
                nc.sync.dma_start(out=tri01, in_=tri_d[:])
                woT_s = persist.tile([128, HPC, dim], f16)
                nc.sync.dma_start(out=woT_s, in_=wo_d[:])

                def finish_rope(raw, store, h, s0):
                    # rot matmul emitted one chain late so the PE never waits
                    # on the scalar-engine raw copy
                    rot = ps1.tile([128, SC], f32, tag="rot")
                    nc.tensor.matmul(rot, lhsT=rTs, rhs=raw, start=True, stop=True)
                    t1 = p1t.tile([128, SC], f16, tag="t1")
                    nc.vector.tensor_mul(t1, raw, cosT[:, s0 : s0 + SC])
                    t2 = p1t.tile([128, SC], f16, tag="t2")
                    nc.vector.tensor_mul(t2, rot, sinT[:, s0 : s0 + SC])
                    nc.gpsimd.tensor_add(store[:, h, s0 : s0 + SC], t1, t2)

                pending = None
                for si in range(n_sc):
                    s0 = si * SC
                    if si < 3:
                        xt = xts[si]
                    else:
                        xt = p1x.tile([128, n_din, SC], f16, tag="xt", name="xt3")
                        nc.sync.dma_start(out=xt, in_=x_d[:, si, :, :])

                    for w_s, store in ((wq_s, qT), (wk_s, kT)):
                        for h in range(HPC):
                            acc = ps1.tile([128, SC], f32, tag="acc")
                            for c in range(n_din):
                                nc.tensor.matmul(
                                    acc,
                                    lhsT=w_s[:, c, h * HD : (h + 1) * HD],
                                    rhs=xt[:, c, :],
                                    start=(c == 0),
                                    stop=(c == n_din - 1),
                                )
                            raw = p1t.tile([128, SC], f16, tag="raw", bufs=3)
                            nc.scalar.copy(raw, acc)
                            if pending is not None:
                                finish_rope(*pending)
                            pending = (raw, store, h, s0)

                    for sp in range(SC // 256):   # v chains in psum pairs
                        vacc2 = ps1.tile([128, 2, SC], f32, tag="vacc")
                        for j in range(2):
                            sub = sp * 2 + j
                            for c in range(n_din):
                                nc.tensor.matmul(
                                    vacc2[:, j, :],
                                    lhsT=xt[:, c, sub * 128 : (sub + 1) * 128],
                                    rhs=wv_s[:, c, :],
                                    start=(c == 0),
                                    stop=(c == n_din - 1),
                                )
                        if pending is not None:
                            finish_rope(*pending)
                            pending = None
                        vdst = vS[:, si * 4 + sp * 2 : si * 4 + sp * 2 + 2, :]
                        if sp == 0:
                            nc.scalar.copy(vdst, vacc2)
                        else:
                            nc.vector.tensor_copy(vdst, vacc2)

            # ------------- phase 2+3: attention + output projection -------------
            with (
                tc.tile_pool(name="p2", bufs=6) as p2,
                tc.tile_pool(name="p2l", bufs=2) as p2l,
                tc.tile_pool(name="p2r", bufs=2) as p2r,
                tc.tile_pool(name="p3", bufs=2) as p3,
                tc.tile_pool(name="ps_t", bufs=2, space="PSUM") as ps_t,
                tc.tile_pool(name="ps_o", bufs=2, space="PSUM") as ps_o,
                tc.tile_pool(name="ps_l", bufs=2, space="PSUM") as ps_l,
            ):
                def phase3(qc):
                    # outT[n, s] = woT.T @ uT for q-chunk qc (all heads);
                    # psum pairs + one grouped DMA per 4 row-chunks
                    q0 = qc * SC
                    for og in range(n_oc // 4):
                        ot4 = p3.tile([128, 4, SC], f16, tag="ot")
                        for op_ in range(2):
                            pos2 = ps_t.tile([128, 2, SC], f32, tag="st", name="pos")
                            for j in range(2):
                                oc = og * 4 + op_ * 2 + j
                                for h in range(HPC):
                                    nc.tensor.matmul(
                                        pos2[:, j, :],
                                        lhsT=woT_s[:, h, oc * 128 : (oc + 1) * 128],
                                        rhs=uT[:, h, q0 : q0 + SC],
                                        start=(h == 0),
                                        stop=(h == HPC - 1),
                                    )
                            dst = ot4[:, op_ * 2 : op_ * 2 + 2, :]
                            if op_ == 0:
                                nc.scalar.copy(dst, pos2)
                            else:
                                nc.vector.tensor_copy(dst, pos2)
                        nc.sync.dma_start(
                            out=out_d[:, qc, og * 4 : (og + 1) * 4, :], in_=ot4
                        )

                for qc in range(n_sc):
                    q0 = qc * SC
                    nfull = 4 * qc          # full (sub-diagonal) k-chunks
                    nkc = nfull + 4
                    for h in range(HPC):
                        outp = ps_o.tile([128, SC], f32, tag="o")
                        lrep = ps_l.tile([128, SC], f32, tag="l")
                        first_ones = True
                        pend_av = []        # (kc, pt AP) awaiting AV, in order
                        quad = []           # pts awaiting a denominator quad

                        def flush_av(upto):
                            # AV matmuls lag the score/exp stream to keep exp
                            # latency off the PE critical path
                            nonlocal pend_av
                            while pend_av and len(pend_av) > upto:
                                kc, pt_ap, co = pend_av.pop(0)
                                nc.tensor.matmul(
                                    outp[:, co:],
                                    lhsT=vS[:, kc, h * HD : (h + 1) * HD],
                                    rhs=pt_ap,
                                    start=(kc == 0),
                                    stop=(kc == nkc - 1),
                                )

                        # --- full chunks, exp'd in pairs ---
                        for pr in range(nfull // 2):
                            st2 = ps_t.tile([128, 2, SC], f32, tag="st")
                            pt2 = p2.tile([128, 2, SC], f16, tag="pt")
                            for j in range(2):
                                kc = pr * 2 + j
                                nc.tensor.matmul(
                                    st2[:, j, :],
                                    lhsT=kT[:, h, kc * 128 : (kc + 1) * 128],
                                    rhs=qT[:, h, q0 : q0 + SC],
                                    start=True,
                                    stop=True,
                                )
                            nc.scalar.activation(pt2, st2, EXP, scale=scale)
                            for j in range(2):
                                kc = pr * 2 + j
                                pend_av.append((kc, pt2[:, j, :], 0))
                                quad.append(pt2[:, j, :])
                            if len(quad) == 4:
                                lp = p2l.tile([128, SC], f16, tag="lp")
                                nc.vector.tensor_add(lp, quad[0], quad[1])
                                nc.vector.tensor_add(lp, lp, quad[2])
                                nc.vector.tensor_add(lp, lp, quad[3])
                                nc.tensor.matmul(
                                    lrep, lhsT=ones, rhs=lp,
                                    start=first_ones, stop=False,
                                )
                                first_ones = False
                                quad = []
                            flush_av(2)

                        # --- diagonal chunks, trimmed + masked ---
                        for di in range(4):
                            kc = nfull + di
                            co = 128 * di
                            st2 = ps_t.tile([128, 2, SC], f32, tag="st")
                            pt2 = p2.tile([128, 2, SC], f16, tag="pt")
                            nc.tensor.matmul(
                                st2[:, 0, co:],
                                lhsT=kT[:, h, kc * 128 : (kc + 1) * 128],
                                rhs=qT[:, h, q0 + co : q0 + SC],
                                start=True,
                                stop=True,
                            )
                            nc.scalar.activation(
                                pt2[:, 0, co:], st2[:, 0, co:], EXP, scale=scale
                            )
                            nc.vector.tensor_mul(
                                pt2[:, 0, co : co + 128],
                                pt2[:, 0, co : co + 128],
                                tri01,
                            )
                            nc.tensor.matmul(
                                lrep[:, co:],
                                lhsT=ones,
                                rhs=pt2[:, 0, co:],
                                start=first_ones,
                                stop=(di == 3),
                            )
                            first_ones = False
                            pend_av.append((kc, pt2[:, 0, co:], co))
                            flush_av(2)
                        flush_av(0)

                        rec = p2r.tile([128, SC], f32, tag="rec")
                        nc.vector.reciprocal_approx_fast(rec, lrep)
                        nc.vector.tensor_mul(uT[:, h, q0 : q0 + SC], outp, rec)
                        if h == 0 and qc > 0:
                            phase3(qc - 1)
                phase3(n_sc - 1)

    nc.compile()
    return nc


def make_in_maps(x, Wq, Wk, Wv, Wo):
    cosT, sinT = _rope_tables_T(S, HD)
    rT = _rot_matrix_T(HD)
    ones = np.ones((HD, HD), dtype=np.float16)
    tri = _tri01()
    n_din, n_sc = DIM // 128, S // SC
    xts = []
    for g in range(DP):
        xT = x[g].T.astype(np.float16)                      # [din, s]
        xts.append(np.ascontiguousarray(
            xT.reshape(n_din, 128, n_sc, SC).transpose(1, 2, 0, 3)
        ))                                                  # [128, si, c, j]
    in_maps = []
    for c in range(N_CORES):
        g, r = divmod(c, TP)
        sl = slice(r * DLOC, (r + 1) * DLOC)

        def tile_w(W):
            wT = W[sl, :].T.astype(np.float16)              # [din, dloc]
            return np.ascontiguousarray(
                wT.reshape(n_din, 128, DLOC).transpose(1, 0, 2)
            )

        woT = Wo[:, sl].T.astype(np.float16)                # [dloc, dim]
        wo_t = np.ascontiguousarray(
            woT.reshape(HPC, 128, DIM).transpose(1, 0, 2)
        )
        in_maps.append(
            {
                "x": xts[g],
                "wq": tile_w(Wq),
                "wk": tile_w(Wk),
                "wv": tile_w(Wv),
                "wo": wo_t,
                "cosT": cosT,
                "sinT": sinT,
                "rT": rT,
                "ones": ones,
                "tri": tri,
            }
        )
    return in_maps


def kernel(x, Wq, Wk, Wv, Wo, _trace=False):
    """Full-input / full-output entry point. Shards over 8 cores internally."""
    if "/opt/trn_rl_repo" not in sys.path:
        sys.path.insert(0, "/opt/trn_rl_repo")
    from concourse.bass_utils import run_bass_kernel_spmd

    x = np.asarray(x, dtype=np.float32)
    Wq, Wk, Wv, Wo = (np.asarray(w, dtype=np.float32) for w in (Wq, Wk, Wv, Wo))

    key = (B, S, DIM)
    if key not in _PROGRAM_CACHE:
        _PROGRAM_CACHE[key] = build_program(S, DIM)
    nc = _PROGRAM_CACHE[key]

    in_maps = make_in_maps(x, Wq, Wk, Wv, Wo)
    res = run_bass_kernel_spmd(
        nc, in_maps, core_ids=list(range(N_CORES)), trace=_trace
    )
    kernel.last_results = res
    out = np.empty((B, S, DIM), dtype=np.float32)
    for g in range(DP):
        acc = res.results[g * TP]["out"].astype(np.float32)
        for r in range(1, TP):
            acc = acc + res.results[g * TP + r]["out"].astype(np.float32)
        # [128, qc, oc, j] -> [oc*128, qc*512]
        outT = acc.transpose(2, 0, 1, 3).reshape(DIM, S)
        out[g] = outT.T
    return out


# revision 8
# speedup vs baseline: 1.4315x; 1.0229x over previous
"""Multi-head causal self-attention with RoPE on 8 Trainium2 NeuronCores.

Sharding: DP(2) x TP(4). Cores [4g, 4g+4) own batch g; within a group,
core r owns heads [4r, 4r+4) (rows [r*512,(r+1)*512) of Wq/Wk/Wv and the
matching columns of Wo). The host sums the 4 partial output projections
per batch (replaces the TP all-reduce); partial sums travel as fp16.

Performance notes (measured on TRN2):
  - PE matmul issue rate is N cycles @2.4GHz regardless of operand dtype
    (fp16 == bf16 == weight-reuse); the kernel is PE-streaming-bound, so
    everything else is organized to keep the PE FIFO dense.
  - dma_start issue on the Sync engine costs ~2.7ns per descriptor line;
    all DRAM tensors are pre-tiled on the host so every transfer is 128
    long per-partition-contiguous descriptors (~0.35us issue each).
  - Scalar activations pay a ~370-cycle access-latency adder, so exps are
    batched two k-chunks per call (st pairs span 2 PSUM banks; engines
    other than the PE may read across banks).
  - Softmax denominators: full (sub-diagonal) k-chunks are summed in
    quads on the DVE (fp16, 2x mode) with one ones-matmul per quad;
    diagonal chunks get individual column-trimmed ones-matmuls.
  - Causality: diagonal-band score chunks are column-trimmed to
    N = 512-128*di and masked multiplicatively (after exp) with a single
    [128,128] triangular 0/1 mask.
"""

import sys

import numpy as np

B, S, DIM = 2, 2048, 2048
NUM_HEADS = 16
HD = 128
N_CORES = 8
DP = 2                       # data-parallel groups (one batch each)
TP = N_CORES // DP           # tensor-parallel ranks per group
HPC = NUM_HEADS // TP        # heads per core (4)
DLOC = HPC * HD              # per-core slice of the model dim (512)
ROPE_BASE = 10000.0
SC = 512                     # s-chunk for projections / attention q-chunk

_PROGRAM_CACHE = {}


def _rope_tables_T(seq_len, head_dim):
    # match reference float32 arithmetic: inv_freq over even indices,
    # emb = cat(freqs, freqs); returned transposed [head_dim, seq_len]
    inv_freq = (
        1.0
        / (np.float32(ROPE_BASE)
           ** (np.arange(0, head_dim, 2, dtype=np.float32) / np.float32(head_dim)))
    ).astype(np.float32)
    t = np.arange(seq_len, dtype=np.float32)
    freqs = np.outer(t, inv_freq).astype(np.float32)      # [S, D/2]
    emb = np.concatenate([freqs, freqs], axis=-1)         # [S, D]
    return (
        np.ascontiguousarray(np.cos(emb).astype(np.float16).T),
        np.ascontiguousarray(np.sin(emb).astype(np.float16).T),
    )


def _rot_matrix_T(head_dim):
    # rotated = cat(-x[1::2], x[::2]) = R @ x; return R.T [D, D]
    d2 = head_dim // 2
    R = np.zeros((head_dim, head_dim), dtype=np.float16)
    for dp in range(d2):
        R[dp, 2 * dp + 1] = -1.0
    for dp in range(d2, head_dim):
        R[dp, 2 * (dp - d2)] = 1.0
    return np.ascontiguousarray(R.T)


def _tri01():
    # tri01[kk, qq] = 1 if kk <= qq else 0 (multiplicative causal mask for
    # the [128,128] diagonal block of every diagonal k-chunk)
    kk = np.arange(128)[:, None]
    qq = np.arange(128)[None, :]
    return np.ascontiguousarray((kk <= qq).astype(np.float16))


def build_program(s=S, dim=DIM):
    """Per-core SPMD Bass program (identical on every core)."""
    if "/opt/trn_rl_repo" not in sys.path:
        sys.path.insert(0, "/opt/trn_rl_repo")
    import concourse.bacc as bacc
    import concourse.mybir as mybir
    import concourse.tile as tile

    f32 = mybir.dt.float32
    f16 = mybir.dt.float16
    EXP = mybir.ActivationFunctionType.Exp

    n_din = dim // 128          # contraction chunks for projections (16)
    n_sc = s // SC              # s-chunks (4)
    n_oc = dim // 128           # output-projection row chunks (16)
    scale = float(HD) ** -0.5

    nc = bacc.Bacc("TRN2", target_bir_lowering=False, debug=False)

    # all DRAM tensors pre-tiled on the host: partition dim first, then
    # per-partition-contiguous free dims, so DMAs are 128 fat descriptors
    x_d = nc.dram_tensor("x", [128, n_sc, n_din, SC], f16, kind="ExternalInput")
    wq_d = nc.dram_tensor("wq", [128, n_din, DLOC], f16, kind="ExternalInput")
    wk_d = nc.dram_tensor("wk", [128, n_din, DLOC], f16, kind="ExternalInput")
    wv_d = nc.dram_tensor("wv", [128, n_din, DLOC], f16, kind="ExternalInput")
    wo_d = nc.dram_tensor("wo", [128, HPC, dim], f16, kind="ExternalInput")
    cosT_d = nc.dram_tensor("cosT", [HD, s], f16, kind="ExternalInput")
    sinT_d = nc.dram_tensor("sinT", [HD, s], f16, kind="ExternalInput")
    rT_d = nc.dram_tensor("rT", [HD, HD], f16, kind="ExternalInput")
    ones_d = nc.dram_tensor("ones", [HD, HD], f16, kind="ExternalInput")
    tri_d = nc.dram_tensor("tri", [HD, HD], f16, kind="ExternalInput")
    out_d = nc.dram_tensor("out", [128, n_sc, n_oc, SC], f16, kind="ExternalOutput")

    with tile.TileContext(nc) as tc:
        with tc.tile_pool(name="persist", bufs=1) as persist:
            qT = persist.tile([128, HPC, s], f16)   # roped q, [d, h, s]
            kT = persist.tile([128, HPC, s], f16)
            vS = persist.tile([128, s // 128, DLOC], f16)  # [k, chunk, d]
            uT = persist.tile([128, HPC, s], f16)   # attention out, [d, h, s]

            # ---------------- phase 1: qkv projections + RoPE ----------------
            with (
                tc.tile_pool(name="p1x", bufs=3) as p1x,
                tc.tile_pool(name="p1w", bufs=1) as p1w,
                tc.tile_pool(name="p1t", bufs=2) as p1t,
                tc.tile_pool(name="ps1", bufs=2, space="PSUM") as ps1,
            ):
                # startup criticality order: the first q-chain needs wq and
                # xt0 c-groups in order, the first rot matmul needs rT, the
                # rope muls need cos/sin -- interleave quarter-size pieces so
                # compute starts ~4us in and trails the DMA stream
                xts = [
                    p1x.tile([128, n_din, SC], f16, tag="xt", name=f"xt{si}")
                    for si in range(3)
                ]
                wq_s = p1w.tile([128, n_din, DLOC], f16)
                wk_s = p1w.tile([128, n_din, DLOC], f16)
                wv_s = p1w.tile([128, n_din, DLOC], f16)
                rTs = persist.tile([HD, HD], f16)
                cosT = persist.tile([HD, s], f16)
                sinT = persist.tile([HD, s], f16)
                gq = n_din // 4
                for g0 in range(0, n_din, gq):
                    sl4 = slice(g0, g0 + gq)
                    nc.sync.dma_start(out=xts[0][:, sl4, :], in_=x_d[:, 0, sl4, :])
                    nc.sync.dma_start(out=wq_s[:, sl4, :], in_=wq_d[:, sl4, :])
                    if g0 == 0:
                        nc.sync.dma_start(out=rTs, in_=rT_d[:])
                    elif g0 == gq:
                        nc.sync.dma_start(out=cosT, in_=cosT_d[:])
                        nc.sync.dma_start(out=sinT, in_=sinT_d[:])
                nh = n_din // 2
                nc.sync.dma_start(out=wk_s[:, :nh, :], in_=wk_d[:, :nh, :])
                nc.sync.dma_start(out=wk_s[:, nh:, :], in_=wk_d[:, nh:, :])
                nc.sync.dma_start(out=xts[1][:, :nh, :], in_=x_d[:, 1, :nh, :])
                nc.sync.dma_start(out=xts[1][:, nh:, :], in_=x_d[:, 1, nh:, :])
                wv_s_ = wv_s
                nc.sync.dma_start(out=wv_s_[:, :nh, :], in_=wv_d[:, :nh, :])
                nc.sync.dma_start(out=wv_s_[:, nh:, :], in_=wv_d[:, nh:, :])
                nc.sync.dma_start(out=xts[2][:, :nh, :], in_=x_d[:, 2, :nh, :])
                nc.sync.dma_start(out=xts[2][:, nh:, :], in_=x_d[:, 2, nh:, :])
                ones = persist.tile([HD, HD], f16)
                nc.sync.dma_start(out=ones, in_=ones_d[:])
                tri01 = persist.tile([HD, HD], f16)
                nc.sync.dma_start(out=tri01, in_=tri_d[:])
                woT_s = persist.tile([128, HPC, dim], f16)
                nc.sync.dma_start(out=woT_s, in_=wo_d[:])

                def finish_rope(raw, store, h, s0):
                    # rot matmul emitted one chain late so the PE never waits
                    # on the scalar-engine raw copy
                    rot = ps1.tile([128, SC], f32, tag="rot")
                    nc.tensor.matmul(rot, lhsT=rTs, rhs=raw, start=True, stop=True)
                    t1 = p1t.tile([128, SC], f16, tag="t1")
                    nc.vector.tensor_mul(t1, raw, cosT[:, s0 : s0 + SC])
                    t2 = p1t.tile([128, SC], f16, tag="t2")
                    nc.vector.tensor_mul(t2, rot, sinT[:, s0 : s0 + SC])
                    nc.gpsimd.tensor_add(store[:, h, s0 : s0 + SC], t1, t2)

                pending = None
                for si in range(n_sc):
                    s0 = si * SC
                    if si < 3:
                        xt = xts[si]
                    else:
                        xt = p1x.tile([128, n_din, SC], f16, tag="xt", name="xt3")
                        nc.sync.dma_start(out=xt, in_=x_d[:, si, :, :])

                    for w_s, store in ((wq_s, qT), (wk_s, kT)):
                        for h in range(HPC):
                            acc = ps1.tile([128, SC], f32, tag="acc")
                            for c in range(n_din):
                                nc.tensor.matmul(
                                    acc,
                                    lhsT=w_s[:, c, h * HD : (h + 1) * HD],
                                    rhs=xt[:, c, :],
                                    start=(c == 0),
                                    stop=(c == n_din - 1),
                                )
                            raw = p1t.tile([128, SC], f16, tag="raw", bufs=3)
                            nc.scalar.copy(raw, acc)
                            if pending is not None:
                                finish_rope(*pending)
                            pending = (raw, store, h, s0)

                    for sp in range(SC // 256):   # v chains in psum pairs
                        vacc2 = ps1.tile([128, 2, SC], f32, tag="vacc")
                        for j in range(2):
                            sub = sp * 2 + j
                            for c in range(n_din):
                                nc.tensor.matmul(
                                    vacc2[:, j, :],
                                    lhsT=xt[:, c, sub * 128 : (sub + 1) * 128],
                                    rhs=wv_s[:, c, :],
                                    start=(c == 0),
                                    stop=(c == n_din - 1),
                                )
                        if pending is not None:
                            finish_rope(*pending)
                            pending = None
                        vdst = vS[:, si * 4 + sp * 2 : si * 4 + sp * 2 + 2, :]
                        if sp == 0:
                            nc.scalar.copy(vdst, vacc2)
                        else:
                            nc.vector.tensor_copy(vdst, vacc2)

            # ------------- phase 2+3: attention + output projection -------------
            with (
                tc.tile_pool(name="p2", bufs=6) as p2,
                tc.tile_pool(name="p2l", bufs=2) as p2l,
                tc.tile_pool(name="p2r", bufs=2) as p2r,
                tc.tile_pool(name="p3", bufs=2) as p3,
                tc.tile_pool(name="ps_t", bufs=2, space="PSUM") as ps_t,
                tc.tile_pool(name="ps_o", bufs=2, space="PSUM") as ps_o,
                tc.tile_pool(name="ps_l", bufs=2, space="PSUM") as ps_l,
            ):
                def phase3_og(qc, og):
                    # one 4-row-chunk group of the output projection for
                    # q-chunk qc: 16 PE matmuls + 2 psum-pair copies + 1 DMA.
                    # Interleaved into the next q-chunk's attention chains as
                    # PE filler while the first exps complete.
                    q0 = qc * SC
                    ot4 = p3.tile([128, 4, SC], f16, tag="ot")
                    for op_ in range(2):
                        pos2 = ps_t.tile([128, 2, SC], f32, tag="st", name="pos")
                        for j in range(2):
                            oc = og * 4 + op_ * 2 + j
                            for h in range(HPC):
                                nc.tensor.matmul(
                                    pos2[:, j, :],
                                    lhsT=woT_s[:, h, oc * 128 : (oc + 1) * 128],
                                    rhs=uT[:, h, q0 : q0 + SC],
                                    start=(h == 0),
                                    stop=(h == HPC - 1),
                                )
                        dst = ot4[:, op_ * 2 : op_ * 2 + 2, :]
                        if op_ == 0:
                            nc.scalar.copy(dst, pos2)
                        else:
                            nc.vector.tensor_copy(dst, pos2)
                    nc.sync.dma_start(
                        out=out_d[:, qc, og * 4 : (og + 1) * 4, :], in_=ot4
                    )

                for qc in range(n_sc):
                    q0 = qc * SC
                    nfull = 4 * qc          # full (sub-diagonal) k-chunks
                    nkc = nfull + 4
                    for h in range(HPC):
                        outp = ps_o.tile([128, SC], f32, tag="o")
                        lrep = ps_l.tile([128, SC], f32, tag="l")
                        first_ones = True
                        pend_av = []        # (kc, pt AP) awaiting AV, in order
                        quad = []           # pts awaiting a denominator quad

                        def flush_av(upto):
                            # AV matmuls lag the score/exp stream to keep exp
                            # latency off the PE critical path
                            nonlocal pend_av
                            while pend_av and len(pend_av) > upto:
                                kc, pt_ap, co = pend_av.pop(0)
                                nc.tensor.matmul(
                                    outp[:, co:],
                                    lhsT=vS[:, kc, h * HD : (h + 1) * HD],
                                    rhs=pt_ap,
                                    start=(kc == 0),
                                    stop=(kc == nkc - 1),
                                )

                        # --- full chunks, exp'd in pairs ---
                        for pr in range(nfull // 2):
                            st2 = ps_t.tile([128, 2, SC], f32, tag="st")
                            pt2 = p2.tile([128, 2, SC], f16, tag="pt")
                            for j in range(2):
                                kc = pr * 2 + j
                                nc.tensor.matmul(
                                    st2[:, j, :],
                                    lhsT=kT[:, h, kc * 128 : (kc + 1) * 128],
                                    rhs=qT[:, h, q0 : q0 + SC],
                                    start=True,
                                    stop=True,
                                )
                            nc.scalar.activation(pt2, st2, EXP, scale=scale)
                            if pr == 0 and qc > 0:
                                # PE filler while the first exps complete:
                                # previous q-chunk's output projection group
                                phase3_og(qc - 1, h)
                            for j in range(2):
                                kc = pr * 2 + j
                                pend_av.append((kc, pt2[:, j, :], 0))
                                quad.append(pt2[:, j, :])
                            if len(quad) == 4:
                                lp = p2l.tile([128, SC], f16, tag="lp")
                                nc.vector.tensor_add(lp, quad[0], quad[1])
                                nc.vector.tensor_add(lp, lp, quad[2])
                                nc.vector.tensor_add(lp, lp, quad[3])
                                nc.tensor.matmul(
                                    lrep, lhsT=ones, rhs=lp,
                                    start=first_ones, stop=False,
                                )
                                first_ones = False
                                quad = []
                            flush_av(2)

                        # --- diagonal chunks, trimmed + masked ---
                        for di in range(4):
                            kc = nfull + di
                            co = 128 * di
                            st2 = ps_t.tile([128, 2, SC], f32, tag="st")
                            pt2 = p2.tile([128, 2, SC], f16, tag="pt")
                            nc.tensor.matmul(
                                st2[:, 0, co:],
                                lhsT=kT[:, h, kc * 128 : (kc + 1) * 128],
                                rhs=qT[:, h, q0 + co : q0 + SC],
                                start=True,
                                stop=True,
                            )
                            nc.scalar.activation(
                                pt2[:, 0, co:], st2[:, 0, co:], EXP, scale=scale
                            )
                            nc.vector.tensor_mul(
                                pt2[:, 0, co : co + 128],
                                pt2[:, 0, co : co + 128],
                                tri01,
                            )
                            nc.tensor.matmul(
                                lrep[:, co:],
                                lhsT=ones,
                                rhs=pt2[:, 0, co:],
                                start=first_ones,
                                stop=(di == 3),
                            )
                            first_ones = False
                            pend_av.append((kc, pt2[:, 0, co:], co))
                            flush_av(2)
                        flush_av(0)

                        rec = p2r.tile([128, SC], f32, tag="rec")
                        nc.vector.reciprocal_approx_fast(rec, lrep)
                        nc.vector.tensor_mul(uT[:, h, q0 : q0 + SC], outp, rec)
                for og in range(n_oc // 4):
                    phase3_og(n_sc - 1, og)

    nc.compile()
    return nc


def make_in_maps(x, Wq, Wk, Wv, Wo):
    cosT, sinT = _rope_tables_T(S, HD)
    rT = _rot_matrix_T(HD)
    ones = np.ones((HD, HD), dtype=np.float16)
    tri = _tri01()
    n_din, n_sc = DIM // 128, S // SC
    xts = []
    for g in range(DP):
        xT = x[g].T.astype(np.float16)                      # [din, s]
        xts.append(np.ascontiguousarray(
            xT.reshape(n_din, 128, n_sc, SC).transpose(1, 2, 0, 3)
        ))                                                  # [128, si, c, j]
    in_maps = []
    for c in range(N_CORES):
        g, r = divmod(c, TP)
        sl = slice(r * DLOC, (r + 1) * DLOC)

        def tile_w(W):
            wT = W[sl, :].T.astype(np.float16)              # [din, dloc]
            return np.ascontiguousarray(
                wT.reshape(n_din, 128, DLOC).transpose(1, 0, 2)
            )

        woT = Wo[:, sl].T.astype(np.float16)                # [dloc, dim]
        wo_t = np.ascontiguousarray(
            woT.reshape(HPC, 128, DIM).transpose(1, 0, 2)
        )
        in_maps.append(
            {
                "x": xts[g],
                "wq": tile_w(Wq),
                "wk": tile_w(Wk),
                "wv": tile_w(Wv),
                "wo": wo_t,
                "cosT": cosT,
                "sinT": sinT,
                "rT": rT,
                "ones": ones,
                "tri": tri,
            }
        )
    return in_maps


def kernel(x, Wq, Wk, Wv, Wo, _trace=False):
    """Full-input / full-output entry point. Shards over 8 cores internally."""
    if "/opt/trn_rl_repo" not in sys.path:
        sys.path.insert(0, "/opt/trn_rl_repo")
    from concourse.bass_utils import run_bass_kernel_spmd

    x = np.asarray(x, dtype=np.float32)
    Wq, Wk, Wv, Wo = (np.asarray(w, dtype=np.float32) for w in (Wq, Wk, Wv, Wo))

    key = (B, S, DIM)
    if key not in _PROGRAM_CACHE:
        _PROGRAM_CACHE[key] = build_program(S, DIM)
    nc = _PROGRAM_CACHE[key]

    in_maps = make_in_maps(x, Wq, Wk, Wv, Wo)
    res = run_bass_kernel_spmd(
        nc, in_maps, core_ids=list(range(N_CORES)), trace=_trace
    )
    kernel.last_results = res
    out = np.empty((B, S, DIM), dtype=np.float32)
    for g in range(DP):
        acc = res.results[g * TP]["out"].astype(np.float32)
        for r in range(1, TP):
            acc = acc + res.results[g * TP + r]["out"].astype(np.float32)
        # [128, qc, oc, j] -> [oc*128, qc*512]
        outT = acc.transpose(2, 0, 1, 3).reshape(DIM, S)
        out[g] = outT.T
    return out


# revision 14
# speedup vs baseline: 1.4459x; 1.0100x over previous
"""Multi-head causal self-attention with RoPE on 8 Trainium2 NeuronCores.

Sharding: DP(2) x TP(4). Cores [4g, 4g+4) own batch g; within a group,
core r owns heads [4r, 4r+4) (rows [r*512,(r+1)*512) of Wq/Wk/Wv and the
matching columns of Wo). The host sums the 4 partial output projections
per batch (replaces the TP all-reduce); partial sums travel as fp16.

Performance notes (measured on TRN2):
  - PE matmul issue rate is N cycles @2.4GHz regardless of operand dtype
    (fp16 == bf16 == weight-reuse); the kernel is PE-streaming-bound, so
    everything else is organized to keep the PE FIFO dense.
  - dma_start issue on the Sync engine costs ~2.7ns per descriptor line;
    all DRAM tensors are pre-tiled on the host so every transfer is 128
    long per-partition-contiguous descriptors (~0.35us issue each).
  - Scalar activations pay a ~370-cycle access-latency adder, so exps are
    batched two k-chunks per call (st pairs span 2 PSUM banks; engines
    other than the PE may read across banks).
  - Softmax denominators: full (sub-diagonal) k-chunks are summed in
    quads on the DVE (fp16, 2x mode) with one ones-matmul per quad;
    diagonal chunks get individual column-trimmed ones-matmuls.
  - Causality: diagonal-band score chunks are column-trimmed to
    N = 512-128*di and masked multiplicatively (after exp) with a single
    [128,128] triangular 0/1 mask.
"""

import sys

import numpy as np

B, S, DIM = 2, 2048, 2048
NUM_HEADS = 16
HD = 128
N_CORES = 8
DP = 2                       # data-parallel groups (one batch each)
TP = N_CORES // DP           # tensor-parallel ranks per group
HPC = NUM_HEADS // TP        # heads per core (4)
DLOC = HPC * HD              # per-core slice of the model dim (512)
ROPE_BASE = 10000.0
SC = 512                     # s-chunk for projections / attention q-chunk

_PROGRAM_CACHE = {}


def _rope_tables_T(seq_len, head_dim):
    # match reference float32 arithmetic: inv_freq over even indices,
    # emb = cat(freqs, freqs); returned transposed [head_dim, seq_len]
    inv_freq = (
        1.0
        / (np.float32(ROPE_BASE)
           ** (np.arange(0, head_dim, 2, dtype=np.float32) / np.float32(head_dim)))
    ).astype(np.float32)
    t = np.arange(seq_len, dtype=np.float32)
    freqs = np.outer(t, inv_freq).astype(np.float32)      # [S, D/2]
    emb = np.concatenate([freqs, freqs], axis=-1)         # [S, D]
    return (
        np.ascontiguousarray(np.cos(emb).astype(np.float16).T),
        np.ascontiguousarray(np.sin(emb).astype(np.float16).T),
    )


def _rot_matrix_T(head_dim):
    # rotated = cat(-x[1::2], x[::2]) = R @ x; return R.T [D, D]
    d2 = head_dim // 2
    R = np.zeros((head_dim, head_dim), dtype=np.float16)
    for dp in range(d2):
        R[dp, 2 * dp + 1] = -1.0
    for dp in range(d2, head_dim):
        R[dp, 2 * (dp - d2)] = 1.0
    return np.ascontiguousarray(R.T)


def _tri01():
    # tri01[kk, qq] = 1 if kk <= qq else 0 (multiplicative causal mask for
    # the [128,128] diagonal block of every diagonal k-chunk)
    kk = np.arange(128)[:, None]
    qq = np.arange(128)[None, :]
    return np.ascontiguousarray((kk <= qq).astype(np.float16))


def build_program(s=S, dim=DIM):
    """Per-core SPMD Bass program (identical on every core)."""
    if "/opt/trn_rl_repo" not in sys.path:
        sys.path.insert(0, "/opt/trn_rl_repo")
    import concourse.bacc as bacc
    import concourse.mybir as mybir
    import concourse.tile as tile

    f32 = mybir.dt.float32
    f16 = mybir.dt.float16
    EXP = mybir.ActivationFunctionType.Exp

    n_din = dim // 128          # contraction chunks for projections (16)
    n_sc = s // SC              # s-chunks (4)
    n_oc = dim // 128           # output-projection row chunks (16)
    scale = float(HD) ** -0.5

    nc = bacc.Bacc("TRN2", target_bir_lowering=False, debug=False)

    # all DRAM tensors pre-tiled on the host: partition dim first, then
    # per-partition-contiguous free dims, so DMAs are 128 fat descriptors
    x_d = nc.dram_tensor("x", [128, n_sc, n_din, SC], f16, kind="ExternalInput")
    wq_d = nc.dram_tensor("wq", [128, HPC, n_din, HD], f16, kind="ExternalInput")
    wk_d = nc.dram_tensor("wk", [128, HPC, n_din, HD], f16, kind="ExternalInput")
    wv_d = nc.dram_tensor("wv", [128, n_din, DLOC], f16, kind="ExternalInput")
    wo_d = nc.dram_tensor("wo", [128, HPC, dim], f16, kind="ExternalInput")
    cosT_d = nc.dram_tensor("cosT", [HD, s], f16, kind="ExternalInput")
    sinT_d = nc.dram_tensor("sinT", [HD, s], f16, kind="ExternalInput")
    rT_d = nc.dram_tensor("rT", [HD, HD], f16, kind="ExternalInput")
    ones_d = nc.dram_tensor("ones", [HD, HD], f16, kind="ExternalInput")
    tri_d = nc.dram_tensor("tri", [HD, HD], f16, kind="ExternalInput")
    out_d = nc.dram_tensor("out", [128, n_sc, n_oc, SC], f16, kind="ExternalOutput")

    with tile.TileContext(nc) as tc:
        with tc.tile_pool(name="persist", bufs=1) as persist:
            qT = persist.tile([128, HPC, s], f16)   # roped q, [d, h, s]
            kT = persist.tile([128, HPC, s], f16)
            vS = persist.tile([128, s // 128, DLOC], f16)  # [k, chunk, d]
            uT = persist.tile([128, HPC, s], f16)   # attention out, [d, h, s]

            # ---------------- phase 1: qkv projections + RoPE ----------------
            with (
                tc.tile_pool(name="p1x", bufs=3) as p1x,
                tc.tile_pool(name="p1w", bufs=1) as p1w,
                tc.tile_pool(name="p1t", bufs=2) as p1t,
                tc.tile_pool(name="ps1", bufs=2, space="PSUM") as ps1,
            ):
                # startup criticality order: chain (q,h=0) is paced by xt0's
                # c-groups and wq's h=0 slice; later pieces arrive while
                # earlier chains compute
                xts = [
                    p1x.tile([128, n_din, SC], f16, tag="xt", name=f"xt{si}")
                    for si in range(3)
                ]
                wq_s = p1w.tile([128, HPC, n_din, HD], f16)
                wk_s = p1w.tile([128, HPC, n_din, HD], f16)
                wv_s = p1w.tile([128, n_din, DLOC], f16)
                rTs = persist.tile([HD, HD], f16)
                cosT = persist.tile([HD, s], f16)
                sinT = persist.tile([HD, s], f16)
                gq = n_din // 4
                nc.sync.dma_start(out=xts[0][:, :gq, :], in_=x_d[:, 0, :gq, :])
                nc.sync.dma_start(out=wq_s[:, 0, :, :], in_=wq_d[:, 0, :, :])
                for g0 in range(gq, n_din, gq):
                    sl4 = slice(g0, g0 + gq)
                    nc.sync.dma_start(out=xts[0][:, sl4, :], in_=x_d[:, 0, sl4, :])
                for h in range(1, HPC):
                    nc.sync.dma_start(out=wq_s[:, h, :, :], in_=wq_d[:, h, :, :])
                nc.sync.dma_start(out=rTs, in_=rT_d[:])
                nc.sync.dma_start(out=cosT, in_=cosT_d[:])
                nc.sync.dma_start(out=sinT, in_=sinT_d[:])
                for h in range(HPC):
                    nc.sync.dma_start(out=wk_s[:, h, :, :], in_=wk_d[:, h, :, :])
                nh = n_din // 2
                nc.sync.dma_start(out=xts[1][:, :nh, :], in_=x_d[:, 1, :nh, :])
                nc.sync.dma_start(out=xts[1][:, nh:, :], in_=x_d[:, 1, nh:, :])
                nc.sync.dma_start(out=wv_s[:, :nh, :], in_=wv_d[:, :nh, :])
                nc.sync.dma_start(out=wv_s[:, nh:, :], in_=wv_d[:, nh:, :])
                nc.sync.dma_start(out=xts[2][:, :nh, :], in_=x_d[:, 2, :nh, :])
                nc.sync.dma_start(out=xts[2][:, nh:, :], in_=x_d[:, 2, nh:, :])
                ones = persist.tile([HD, HD], f16)
                nc.sync.dma_start(out=ones, in_=ones_d[:])
                tri01 = persist.tile([HD, HD], f16)
                nc.sync.dma_start(out=tri01, in_=tri_d[:])
                woT_s = persist.tile([128, HPC, dim], f16)
                nc.sync.dma_start(out=woT_s, in_=wo_d[:])

                def finish_rope(raw, store, h, s0):
                    # rot matmul emitted one chain late so the PE never waits
                    # on the scalar-engine raw copy
                    rot = ps1.tile([128, SC], f32, tag="rot")
                    nc.tensor.matmul(rot, lhsT=rTs, rhs=raw, start=True, stop=True)
                    t1 = p1t.tile([128, SC], f16, tag="t1")
                    nc.vector.tensor_mul(t1, raw, cosT[:, s0 : s0 + SC])
                    t2 = p1t.tile([128, SC], f16, tag="t2")
                    nc.vector.tensor_mul(t2, rot, sinT[:, s0 : s0 + SC])
                    nc.gpsimd.tensor_add(store[:, h, s0 : s0 + SC], t1, t2)

                pending = None
                for si in range(n_sc):
                    s0 = si * SC
                    if si < 3:
                        xt = xts[si]
                    else:
                        xt = p1x.tile([128, n_din, SC], f16, tag="xt", name="xt3")
                        nc.sync.dma_start(out=xt, in_=x_d[:, si, :, :])

                    for w_s, store in ((wq_s, qT), (wk_s, kT)):
                        for h in range(HPC):
                            acc = ps1.tile([128, SC], f32, tag="acc")
                            for c in range(n_din):
                                nc.tensor.matmul(
                                    acc,
                                    lhsT=w_s[:, h, c, :],
                                    rhs=xt[:, c, :],
                                    start=(c == 0),
                                    stop=(c == n_din - 1),
                                )
                            raw = p1t.tile([128, SC], f16, tag="raw", bufs=3)
                            nc.scalar.copy(raw, acc)
                            if pending is not None:
                                finish_rope(*pending)
                            pending = (raw, store, h, s0)

                    for sp in range(SC // 256):   # v chains in psum pairs
                        vacc2 = ps1.tile([128, 2, SC], f32, tag="vacc")
                        for j in range(2):
                            sub = sp * 2 + j
                            for c in range(n_din):
                                nc.tensor.matmul(
                                    vacc2[:, j, :],
                                    lhsT=xt[:, c, sub * 128 : (sub + 1) * 128],
                                    rhs=wv_s[:, c, :],
                                    start=(c == 0),
                                    stop=(c == n_din - 1),
                                )
                        if pending is not None:
                            finish_rope(*pending)
                            pending = None
                        vdst = vS[:, si * 4 + sp * 2 : si * 4 + sp * 2 + 2, :]
                        if sp == 0:
                            nc.scalar.copy(vdst, vacc2)
                        else:
                            nc.vector.tensor_copy(vdst, vacc2)

            # ------------- phase 2+3: attention + output projection -------------
            with (
                tc.tile_pool(name="p2", bufs=6) as p2,
                tc.tile_pool(name="p2l", bufs=2) as p2l,
                tc.tile_pool(name="p2r", bufs=2) as p2r,
                tc.tile_pool(name="p3", bufs=2) as p3,
                tc.tile_pool(name="ps_t", bufs=3, space="PSUM") as ps_t,
                tc.tile_pool(name="ps_o", bufs=2, space="PSUM") as ps_o,
            ):
                def phase3_og(qc, og):
                    # one 4-row-chunk group of the output projection for
                    # q-chunk qc: 16 PE matmuls + 2 psum-pair copies + 1 DMA.
                    # Interleaved into the next q-chunk's attention chains as
                    # PE filler while the first exps complete.
                    q0 = qc * SC
                    ot4 = p3.tile([128, 4, SC], f16, tag="ot")
                    for op_ in range(2):
                        pos2 = ps_t.tile([128, 2, SC], f32, tag="st", name="pos")
                        for j in range(2):
                            oc = og * 4 + op_ * 2 + j
                            for h in range(HPC):
                                nc.tensor.matmul(
                                    pos2[:, j, :],
                                    lhsT=woT_s[:, h, oc * 128 : (oc + 1) * 128],
                                    rhs=uT[:, h, q0 : q0 + SC],
                                    start=(h == 0),
                                    stop=(h == HPC - 1),
                                )
                        dst = ot4[:, op_ * 2 : op_ * 2 + 2, :]
                        if op_ == 0:
                            nc.scalar.copy(dst, pos2)
                        else:
                            nc.vector.tensor_copy(dst, pos2)
                    nc.sync.dma_start(
                        out=out_d[:, qc, og * 4 : (og + 1) * 4, :], in_=ot4
                    )

                # attention q-chunks processed in rotated order so every
                # chain (including the short qc=0 ones) carries an output-
                # projection filler group from the previously finished chunk
                qc_order = list(range(1, n_sc)) + [0]
                for oi, qc in enumerate(qc_order):
                    q0 = qc * SC
                    nfull = 4 * qc          # full (sub-diagonal) k-chunks
                    nkc = nfull + 4
                    prev_qc = qc_order[oi - 1] if oi > 0 else None
                    for h in range(HPC):
                        outp = ps_o.tile([128, SC], f32, tag="o")
                        lall = p2l.tile([128, SC], f16, tag="lp")
                        lst = [False]       # lall initialized?
                        pend_av = []        # (kc, pt AP, co) awaiting AV
                        filler = [prev_qc] if prev_qc is not None else []

                        def lacc(ap, co):
                            # fp16 DVE accumulation of the softmax denominator
                            if not lst[0]:
                                nc.vector.tensor_copy(lall, ap)
                                lst[0] = True
                            else:
                                nc.vector.tensor_add(
                                    lall[:, co:], lall[:, co:], ap
                                )

                        def flush_av(upto):
                            # AV matmuls lag the score/exp stream to keep exp
                            # latency off the PE critical path
                            while len(pend_av) > upto:
                                kc, pt_ap, co = pend_av.pop(0)
                                nc.tensor.matmul(
                                    outp[:, co:],
                                    lhsT=vS[:, kc, h * HD : (h + 1) * HD],
                                    rhs=pt_ap,
                                    start=(kc == 0),
                                    stop=(kc == nkc - 1),
                                )

                        # --- full chunks, exp'd in pairs ---
                        for pr in range(nfull // 2):
                            st2 = ps_t.tile([128, 2, SC], f32, tag="st")
                            pt2 = p2.tile([128, 2, SC], f16, tag="pt")
                            for j in range(2):
                                kc = pr * 2 + j
                                nc.tensor.matmul(
                                    st2[:, j, :],
                                    lhsT=kT[:, h, kc * 128 : (kc + 1) * 128],
                                    rhs=qT[:, h, q0 : q0 + SC],
                                    start=True,
                                    stop=True,
                                )
                            nc.scalar.activation(pt2, st2, EXP, scale=scale)
                            if filler:
                                # PE filler while the first exps complete
                                phase3_og(filler.pop(), h)
                            if lst[0]:
                                nc.vector.tensor_add(lall, lall, pt2[:, 0, :])
                            else:
                                nc.vector.tensor_add(
                                    lall, pt2[:, 0, :], pt2[:, 1, :]
                                )
                                lst[0] = True
                            if pr > 0:
                                nc.vector.tensor_add(lall, lall, pt2[:, 1, :])
                            pend_av.append((pr * 2, pt2[:, 0, :], 0))
                            pend_av.append((pr * 2 + 1, pt2[:, 1, :], 0))
                            flush_av(2)

                        # --- diagonal chunks, trimmed + masked ---
                        for di in range(4):
                            kc = nfull + di
                            co = 128 * di
                            st2 = ps_t.tile([128, 2, SC], f32, tag="st")
                            pt2 = p2.tile([128, 2, SC], f16, tag="pt")
                            nc.tensor.matmul(
                                st2[:, 0, co:],
                                lhsT=kT[:, h, kc * 128 : (kc + 1) * 128],
                                rhs=qT[:, h, q0 + co : q0 + SC],
                                start=True,
                                stop=True,
                            )
                            nc.scalar.activation(
                                pt2[:, 0, co:], st2[:, 0, co:], EXP, scale=scale
                            )
                            if di == 0 and filler:
                                phase3_og(filler.pop(), h)
                            nc.vector.tensor_mul(
                                pt2[:, 0, co : co + 128],
                                pt2[:, 0, co : co + 128],
                                tri01,
                            )
                            lacc(pt2[:, 0, co:], co)
                            pend_av.append((kc, pt2[:, 0, co:], co))
                            flush_av(2)
                        flush_av(0)

                        # single partition-reduce matmul for the denominator
                        lrep = ps_t.tile([128, 2, SC], f32, tag="st", name="lrep")
                        nc.tensor.matmul(
                            lrep[:, 0, :], lhsT=ones, rhs=lall,
                            start=True, stop=True,
                        )
                        rec = p2r.tile([128, SC], f32, tag="rec")
                        nc.vector.reciprocal_approx_fast(rec, lrep[:, 0, :])
                        nc.vector.tensor_mul(uT[:, h, q0 : q0 + SC], outp, rec)
                for og in range(n_oc // 4):
                    phase3_og(qc_order[-1], og)

    nc.compile()
    return nc


def make_in_maps(x, Wq, Wk, Wv, Wo):
    cosT, sinT = _rope_tables_T(S, HD)
    rT = _rot_matrix_T(HD)
    ones = np.ones((HD, HD), dtype=np.float16)
    tri = _tri01()
    n_din, n_sc = DIM // 128, S // SC
    xts = []
    for g in range(DP):
        xT = x[g].T.astype(np.float16)                      # [din, s]
        xts.append(np.ascontiguousarray(
            xT.reshape(n_din, 128, n_sc, SC).transpose(1, 2, 0, 3)
        ))                                                  # [128, si, c, j]
    in_maps = []
    for c in range(N_CORES):
        g, r = divmod(c, TP)
        sl = slice(r * DLOC, (r + 1) * DLOC)

        def tile_w_h(W):
            # [p, h, c, d] = W.T[c*128+p, h*128+d]
            wT = W[sl, :].T.astype(np.float16)              # [din, dloc]
            return np.ascontiguousarray(
                wT.reshape(n_din, 128, HPC, HD).transpose(1, 2, 0, 3)
            )

        def tile_w_c(W):
            wT = W[sl, :].T.astype(np.float16)              # [din, dloc]
            return np.ascontiguousarray(
                wT.reshape(n_din, 128, DLOC).transpose(1, 0, 2)
            )

        woT = Wo[:, sl].T.astype(np.float16)                # [dloc, dim]
        wo_t = np.ascontiguousarray(
            woT.reshape(HPC, 128, DIM).transpose(1, 0, 2)
        )
        in_maps.append(
            {
                "x": xts[g],
                "wq": tile_w_h(Wq),
                "wk": tile_w_h(Wk),
                "wv": tile_w_c(Wv),
                "wo": wo_t,
                "cosT": cosT,
                "sinT": sinT,
                "rT": rT,
                "ones": ones,
                "tri": tri,
            }
        )
    return in_maps


def kernel(x, Wq, Wk, Wv, Wo, _trace=False):
    """Full-input / full-output entry point. Shards over 8 cores internally."""
    if "/opt/trn_rl_repo" not in sys.path:
        sys.path.insert(0, "/opt/trn_rl_repo")
    from concourse.bass_utils import run_bass_kernel_spmd

    x = np.asarray(x, dtype=np.float32)
    Wq, Wk, Wv, Wo = (np.asarray(w, dtype=np.float32) for w in (Wq, Wk, Wv, Wo))

    key = (B, S, DIM)
    if key not in _PROGRAM_CACHE:
        _PROGRAM_CACHE[key] = build_program(S, DIM)
    nc = _PROGRAM_CACHE[key]

    in_maps = make_in_maps(x, Wq, Wk, Wv, Wo)
    res = run_bass_kernel_spmd(
        nc, in_maps, core_ids=list(range(N_CORES)), trace=_trace
    )
    kernel.last_results = res
    out = np.empty((B, S, DIM), dtype=np.float32)
    for g in range(DP):
        acc = res.results[g * TP]["out"].astype(np.float32)
        for r in range(1, TP):
            acc = acc + res.results[g * TP + r]["out"].astype(np.float32)
        # [128, qc, oc, j] -> [oc*128, qc*512]
        outT = acc.transpose(2, 0, 1, 3).reshape(DIM, S)
        out[g] = outT.T
    return out


# revision 15
# speedup vs baseline: 1.4535x; 1.0052x over previous
"""Multi-head causal self-attention with RoPE on 8 Trainium2 NeuronCores.

Sharding: DP(2) x TP(4). Cores [4g, 4g+4) own batch g; within a group,
core r owns heads [4r, 4r+4) (rows [r*512,(r+1)*512) of Wq/Wk/Wv and the
matching columns of Wo). The host sums the 4 partial output projections
per batch (replaces the TP all-reduce); partial sums travel as fp16.

Performance notes (measured on TRN2):
  - PE matmul issue rate is N cycles @2.4GHz regardless of operand dtype
    (fp16 == bf16 == weight-reuse); the kernel is PE-streaming-bound, so
    everything else is organized to keep the PE FIFO dense.
  - dma_start issue on the Sync engine costs ~2.7ns per descriptor line;
    all DRAM tensors are pre-tiled on the host so every transfer is 128
    long per-partition-contiguous descriptors (~0.35us issue each).
  - Scalar activations pay a ~370-cycle access-latency adder, so exps are
    batched two k-chunks per call (st pairs span 2 PSUM banks; engines
    other than the PE may read across banks).
  - Softmax denominators: full (sub-diagonal) k-chunks are summed in
    quads on the DVE (fp16, 2x mode) with one ones-matmul per quad;
    diagonal chunks get individual column-trimmed ones-matmuls.
  - Causality: diagonal-band score chunks are column-trimmed to
    N = 512-128*di and masked multiplicatively (after exp) with a single
    [128,128] triangular 0/1 mask.
"""

import sys

import numpy as np

B, S, DIM = 2, 2048, 2048
NUM_HEADS = 16
HD = 128
N_CORES = 8
DP = 2                       # data-parallel groups (one batch each)
TP = N_CORES // DP           # tensor-parallel ranks per group
HPC = NUM_HEADS // TP        # heads per core (4)
DLOC = HPC * HD              # per-core slice of the model dim (512)
ROPE_BASE = 10000.0
SC = 512                     # s-chunk for projections / attention q-chunk

_PROGRAM_CACHE = {}


def _rope_tables_T(seq_len, head_dim):
    # match reference float32 arithmetic: inv_freq over even indices,
    # emb = cat(freqs, freqs); returned transposed [head_dim, seq_len]
    inv_freq = (
        1.0
        / (np.float32(ROPE_BASE)
           ** (np.arange(0, head_dim, 2, dtype=np.float32) / np.float32(head_dim)))
    ).astype(np.float32)
    t = np.arange(seq_len, dtype=np.float32)
    freqs = np.outer(t, inv_freq).astype(np.float32)      # [S, D/2]
    emb = np.concatenate([freqs, freqs], axis=-1)         # [S, D]
    return (
        np.ascontiguousarray(np.cos(emb).astype(np.float16).T),
        np.ascontiguousarray(np.sin(emb).astype(np.float16).T),
    )


def _rot_matrix_T(head_dim):
    # rotated = cat(-x[1::2], x[::2]) = R @ x; return R.T [D, D]
    d2 = head_dim // 2
    R = np.zeros((head_dim, head_dim), dtype=np.float16)
    for dp in range(d2):
        R[dp, 2 * dp + 1] = -1.0
    for dp in range(d2, head_dim):
        R[dp, 2 * (dp - d2)] = 1.0
    return np.ascontiguousarray(R.T)


def _tri01():
    # tri01[kk, qq] = 1 if kk <= qq else 0 (multiplicative causal mask for
    # the [128,128] diagonal block of every diagonal k-chunk)
    kk = np.arange(128)[:, None]
    qq = np.arange(128)[None, :]
    return np.ascontiguousarray((kk <= qq).astype(np.float16))


def build_program(s=S, dim=DIM):
    """Per-core SPMD Bass program (identical on every core)."""
    if "/opt/trn_rl_repo" not in sys.path:
        sys.path.insert(0, "/opt/trn_rl_repo")
    import concourse.bacc as bacc
    import concourse.mybir as mybir
    import concourse.tile as tile

    f32 = mybir.dt.float32
    f16 = mybir.dt.float16
    EXP = mybir.ActivationFunctionType.Exp

    n_din = dim // 128          # contraction chunks for projections (16)
    n_sc = s // SC              # s-chunks (4)
    n_oc = dim // 128           # output-projection row chunks (16)
    scale = float(HD) ** -0.5

    nc = bacc.Bacc("TRN2", target_bir_lowering=False, debug=False)

    # all DRAM tensors pre-tiled on the host: partition dim first, then
    # per-partition-contiguous free dims, so DMAs are 128 fat descriptors
    x_d = nc.dram_tensor("x", [128, n_sc, n_din, SC], f16, kind="ExternalInput")
    wq_d = nc.dram_tensor("wq", [128, HPC, n_din, HD], f16, kind="ExternalInput")
    wk_d = nc.dram_tensor("wk", [128, HPC, n_din, HD], f16, kind="ExternalInput")
    wv_d = nc.dram_tensor("wv", [128, n_din, DLOC], f16, kind="ExternalInput")
    wo_d = nc.dram_tensor("wo", [128, HPC, dim], f16, kind="ExternalInput")
    cosT_d = nc.dram_tensor("cosT", [HD, s], f16, kind="ExternalInput")
    sinT_d = nc.dram_tensor("sinT", [HD, s], f16, kind="ExternalInput")
    rT_d = nc.dram_tensor("rT", [HD, HD], f16, kind="ExternalInput")
    ones_d = nc.dram_tensor("ones", [HD, HD], f16, kind="ExternalInput")
    tri_d = nc.dram_tensor("tri", [HD, HD], f16, kind="ExternalInput")
    out_d = nc.dram_tensor("out", [128, n_sc, n_oc, SC], f16, kind="ExternalOutput")

    with tile.TileContext(nc) as tc:
        with tc.tile_pool(name="persist", bufs=1) as persist:
            qT = persist.tile([128, HPC, s], f16)   # roped q, [d, h, s]
            kT = persist.tile([128, HPC, s], f16)
            vS = persist.tile([128, s // 128, DLOC], f16)  # [k, chunk, d]
            uT = persist.tile([128, HPC, s], f16)   # attention out, [d, h, s]

            # ---------------- phase 1: qkv projections + RoPE ----------------
            with (
                tc.tile_pool(name="p1x", bufs=3) as p1x,
                tc.tile_pool(name="p1w", bufs=1) as p1w,
                tc.tile_pool(name="p1t", bufs=2) as p1t,
                tc.tile_pool(name="ps1", bufs=2, space="PSUM") as ps1,
            ):
                # startup criticality order: chain (q,h=0) is paced by xt0's
                # c-groups and wq's h=0 slice; later pieces arrive while
                # earlier chains compute
                xts = [
                    p1x.tile([128, n_din, SC], f16, tag="xt", name=f"xt{si}")
                    for si in range(3)
                ]
                wq_s = p1w.tile([128, HPC, n_din, HD], f16)
                wk_s = p1w.tile([128, HPC, n_din, HD], f16)
                wv_s = p1w.tile([128, n_din, DLOC], f16)
                rTs = persist.tile([HD, HD], f16)
                cosT = persist.tile([HD, s], f16)
                sinT = persist.tile([HD, s], f16)
                gq = n_din // 4
                nc.sync.dma_start(out=xts[0][:, :gq, :], in_=x_d[:, 0, :gq, :])
                nc.sync.dma_start(out=wq_s[:, 0, :, :], in_=wq_d[:, 0, :, :])
                for g0 in range(gq, n_din, gq):
                    sl4 = slice(g0, g0 + gq)
                    nc.sync.dma_start(out=xts[0][:, sl4, :], in_=x_d[:, 0, sl4, :])
                for h in range(1, HPC):
                    nc.sync.dma_start(out=wq_s[:, h, :, :], in_=wq_d[:, h, :, :])
                nc.sync.dma_start(out=rTs, in_=rT_d[:])
                nc.sync.dma_start(out=cosT, in_=cosT_d[:])
                nc.sync.dma_start(out=sinT, in_=sinT_d[:])
                for h in range(HPC):
                    nc.sync.dma_start(out=wk_s[:, h, :, :], in_=wk_d[:, h, :, :])
                nh = n_din // 2
                nc.sync.dma_start(out=xts[1][:, :nh, :], in_=x_d[:, 1, :nh, :])
                nc.sync.dma_start(out=xts[1][:, nh:, :], in_=x_d[:, 1, nh:, :])
                nc.sync.dma_start(out=wv_s[:, :nh, :], in_=wv_d[:, :nh, :])
                nc.sync.dma_start(out=wv_s[:, nh:, :], in_=wv_d[:, nh:, :])
                nc.sync.dma_start(out=xts[2][:, :nh, :], in_=x_d[:, 2, :nh, :])
                nc.sync.dma_start(out=xts[2][:, nh:, :], in_=x_d[:, 2, nh:, :])
                ones = persist.tile([HD, HD], f16)
                nc.sync.dma_start(out=ones, in_=ones_d[:])
                tri01 = persist.tile([HD, HD], f16)
                nc.sync.dma_start(out=tri01, in_=tri_d[:])
                woT_s = persist.tile([128, HPC, dim], f16)
                nc.sync.dma_start(out=woT_s, in_=wo_d[:])

                def finish_rope(raw, store, h, s0):
                    # rot matmul emitted one chain late so the PE never waits
                    # on the scalar-engine raw copy
                    rot = ps1.tile([128, SC], f32, tag="rot")
                    nc.tensor.matmul(rot, lhsT=rTs, rhs=raw, start=True, stop=True)
                    t1 = p1t.tile([128, SC], f16, tag="t1")
                    nc.vector.tensor_mul(t1, raw, cosT[:, s0 : s0 + SC])
                    t2 = p1t.tile([128, SC], f16, tag="t2")
                    nc.vector.tensor_mul(t2, rot, sinT[:, s0 : s0 + SC])
                    nc.gpsimd.tensor_add(store[:, h, s0 : s0 + SC], t1, t2)

                pending = None
                for si in range(n_sc):
                    s0 = si * SC
                    if si < 3:
                        xt = xts[si]
                    else:
                        xt = p1x.tile([128, n_din, SC], f16, tag="xt", name="xt3")
                        nc.sync.dma_start(out=xt, in_=x_d[:, si, :, :])

                    for w_s, store in ((wq_s, qT), (wk_s, kT)):
                        for h in range(HPC):
                            acc = ps1.tile([128, SC], f32, tag="acc")
                            for c in range(n_din):
                                nc.tensor.matmul(
                                    acc,
                                    lhsT=w_s[:, h, c, :],
                                    rhs=xt[:, c, :],
                                    start=(c == 0),
                                    stop=(c == n_din - 1),
                                )
                            raw = p1t.tile([128, SC], f16, tag="raw", bufs=3)
                            nc.scalar.copy(raw, acc)
                            if pending is not None:
                                finish_rope(*pending)
                            pending = (raw, store, h, s0)

                    for sp in range(SC // 256):   # v chains in psum pairs
                        vacc2 = ps1.tile([128, 2, SC], f32, tag="vacc")
                        for j in range(2):
                            sub = sp * 2 + j
                            for c in range(n_din):
                                nc.tensor.matmul(
                                    vacc2[:, j, :],
                                    lhsT=xt[:, c, sub * 128 : (sub + 1) * 128],
                                    rhs=wv_s[:, c, :],
                                    start=(c == 0),
                                    stop=(c == n_din - 1),
                                )
                        if pending is not None:
                            finish_rope(*pending)
                            pending = None
                        vdst = vS[:, si * 4 + sp * 2 : si * 4 + sp * 2 + 2, :]
                        if sp == 0:
                            nc.scalar.copy(vdst, vacc2)
                        else:
                            nc.vector.tensor_copy(vdst, vacc2)

            # ------------- phase 2+3: attention + output projection -------------
            with (
                tc.tile_pool(name="p2", bufs=8) as p2,
                tc.tile_pool(name="p2l", bufs=2) as p2l,
                tc.tile_pool(name="p2r", bufs=2) as p2r,
                tc.tile_pool(name="p3", bufs=2) as p3,
                tc.tile_pool(name="ps_t", bufs=3, space="PSUM") as ps_t,
                tc.tile_pool(name="ps_o", bufs=2, space="PSUM") as ps_o,
            ):
                def phase3_og(qc, og):
                    # one 4-row-chunk group of the output projection for
                    # q-chunk qc: 16 PE matmuls + 2 psum-pair copies + 1 DMA.
                    # Interleaved into the next q-chunk's attention chains as
                    # PE filler while the first exps complete.
                    q0 = qc * SC
                    ot4 = p3.tile([128, 4, SC], f16, tag="ot")
                    for op_ in range(2):
                        pos2 = ps_t.tile([128, 2, SC], f32, tag="st", name="pos")
                        for j in range(2):
                            oc = og * 4 + op_ * 2 + j
                            for h in range(HPC):
                                nc.tensor.matmul(
                                    pos2[:, j, :],
                                    lhsT=woT_s[:, h, oc * 128 : (oc + 1) * 128],
                                    rhs=uT[:, h, q0 : q0 + SC],
                                    start=(h == 0),
                                    stop=(h == HPC - 1),
                                )
                        dst = ot4[:, op_ * 2 : op_ * 2 + 2, :]
                        if op_ == 0:
                            nc.scalar.copy(dst, pos2)
                        else:
                            nc.vector.tensor_copy(dst, pos2)
                    nc.sync.dma_start(
                        out=out_d[:, qc, og * 4 : (og + 1) * 4, :], in_=ot4
                    )

                # attention q-chunks processed in rotated order so every
                # chain (including the short qc=0 ones) carries an output-
                # projection filler group from the previously finished chunk
                qc_order = list(range(1, n_sc)) + [0]
                for oi, qc in enumerate(qc_order):
                    q0 = qc * SC
                    nfull = 4 * qc          # full (sub-diagonal) k-chunks
                    nkc = nfull + 4
                    prev_qc = qc_order[oi - 1] if oi > 0 else None
                    for h in range(HPC):
                        outp = ps_o.tile([128, SC], f32, tag="o")
                        lall = p2l.tile([128, SC], f16, tag="lp")
                        lst = [False]       # lall initialized?
                        pend_av = []        # (kc, pt AP, co) awaiting AV
                        filler = [prev_qc] if prev_qc is not None else []

                        def lacc(ap, co):
                            # fp16 DVE accumulation of the softmax denominator
                            if not lst[0]:
                                nc.vector.tensor_copy(lall, ap)
                                lst[0] = True
                            else:
                                nc.vector.tensor_add(
                                    lall[:, co:], lall[:, co:], ap
                                )

                        def flush_av(upto):
                            # AV matmuls lag the score/exp stream to keep exp
                            # latency off the PE critical path
                            while len(pend_av) > upto:
                                kc, pt_ap, co = pend_av.pop(0)
                                nc.tensor.matmul(
                                    outp[:, co:],
                                    lhsT=vS[:, kc, h * HD : (h + 1) * HD],
                                    rhs=pt_ap,
                                    start=(kc == 0),
                                    stop=(kc == nkc - 1),
                                )

                        # --- full chunks, exp'd in pairs ---
                        for pr in range(nfull // 2):
                            st2 = ps_t.tile([128, 2, SC], f32, tag="st")
                            pt2 = p2.tile([128, 2, SC], f16, tag="pt")
                            for j in range(2):
                                kc = pr * 2 + j
                                nc.tensor.matmul(
                                    st2[:, j, :],
                                    lhsT=kT[:, h, kc * 128 : (kc + 1) * 128],
                                    rhs=qT[:, h, q0 : q0 + SC],
                                    start=True,
                                    stop=True,
                                )
                            nc.scalar.activation(pt2, st2, EXP, scale=scale)
                            if filler:
                                # PE filler while the first exps complete
                                phase3_og(filler.pop(), h)
                            if lst[0]:
                                nc.vector.tensor_add(lall, lall, pt2[:, 0, :])
                            else:
                                nc.vector.tensor_add(
                                    lall, pt2[:, 0, :], pt2[:, 1, :]
                                )
                                lst[0] = True
                            if pr > 0:
                                nc.vector.tensor_add(lall, lall, pt2[:, 1, :])
                            pend_av.append((pr * 2, pt2[:, 0, :], 0))
                            pend_av.append((pr * 2 + 1, pt2[:, 1, :], 0))
                            flush_av(3)

                        # --- diagonal chunks, trimmed + masked ---
                        for di in range(4):
                            kc = nfull + di
                            co = 128 * di
                            st2 = ps_t.tile([128, 2, SC], f32, tag="st")
                            pt2 = p2.tile([128, 2, SC], f16, tag="pt")
                            nc.tensor.matmul(
                                st2[:, 0, co:],
                                lhsT=kT[:, h, kc * 128 : (kc + 1) * 128],
                                rhs=qT[:, h, q0 + co : q0 + SC],
                                start=True,
                                stop=True,
                            )
                            nc.scalar.activation(
                                pt2[:, 0, co:], st2[:, 0, co:], EXP, scale=scale
                            )
                            if di == 0 and filler:
                                phase3_og(filler.pop(), h)
                            nc.vector.tensor_mul(
                                pt2[:, 0, co : co + 128],
                                pt2[:, 0, co : co + 128],
                                tri01,
                            )
                            lacc(pt2[:, 0, co:], co)
                            pend_av.append((kc, pt2[:, 0, co:], co))
                            flush_av(3)
                        flush_av(0)

                        # single partition-reduce matmul for the denominator
                        lrep = ps_t.tile([128, 2, SC], f32, tag="st", name="lrep")
                        nc.tensor.matmul(
                            lrep[:, 0, :], lhsT=ones, rhs=lall,
                            start=True, stop=True,
                        )
                        rec = p2r.tile([128, SC], f32, tag="rec")
                        nc.vector.reciprocal_approx_fast(rec, lrep[:, 0, :])
                        nc.vector.tensor_mul(uT[:, h, q0 : q0 + SC], outp, rec)
                for og in range(n_oc // 4):
                    phase3_og(qc_order[-1], og)

    nc.compile()
    return nc


def make_in_maps(x, Wq, Wk, Wv, Wo):
    cosT, sinT = _rope_tables_T(S, HD)
    rT = _rot_matrix_T(HD)
    ones = np.ones((HD, HD), dtype=np.float16)
    tri = _tri01()
    n_din, n_sc = DIM // 128, S // SC
    xts = []
    for g in range(DP):
        xT = x[g].T.astype(np.float16)                      # [din, s]
        xts.append(np.ascontiguousarray(
            xT.reshape(n_din, 128, n_sc, SC).transpose(1, 2, 0, 3)
        ))                                                  # [128, si, c, j]
    in_maps = []
    for c in range(N_CORES):
        g, r = divmod(c, TP)
        sl = slice(r * DLOC, (r + 1) * DLOC)

        def tile_w_h(W):
            # [p, h, c, d] = W.T[c*128+p, h*128+d]
            wT = W[sl, :].T.astype(np.float16)              # [din, dloc]
            return np.ascontiguousarray(
                wT.reshape(n_din, 128, HPC, HD).transpose(1, 2, 0, 3)
            )

        def tile_w_c(W):
            wT = W[sl, :].T.astype(np.float16)              # [din, dloc]
            return np.ascontiguousarray(
                wT.reshape(n_din, 128, DLOC).transpose(1, 0, 2)
            )

        woT = Wo[:, sl].T.astype(np.float16)                # [dloc, dim]
        wo_t = np.ascontiguousarray(
            woT.reshape(HPC, 128, DIM).transpose(1, 0, 2)
        )
        in_maps.append(
            {
                "x": xts[g],
                "wq": tile_w_h(Wq),
                "wk": tile_w_h(Wk),
                "wv": tile_w_c(Wv),
                "wo": wo_t,
                "cosT": cosT,
                "sinT": sinT,
                "rT": rT,
                "ones": ones,
                "tri": tri,
            }
        )
    return in_maps


def kernel(x, Wq, Wk, Wv, Wo, _trace=False):
    """Full-input / full-output entry point. Shards over 8 cores internally."""
    if "/opt/trn_rl_repo" not in sys.path:
        sys.path.insert(0, "/opt/trn_rl_repo")
    from concourse.bass_utils import run_bass_kernel_spmd

    x = np.asarray(x, dtype=np.float32)
    Wq, Wk, Wv, Wo = (np.asarray(w, dtype=np.float32) for w in (Wq, Wk, Wv, Wo))

    key = (B, S, DIM)
    if key not in _PROGRAM_CACHE:
        _PROGRAM_CACHE[key] = build_program(S, DIM)
    nc = _PROGRAM_CACHE[key]

    in_maps = make_in_maps(x, Wq, Wk, Wv, Wo)
    res = run_bass_kernel_spmd(
        nc, in_maps, core_ids=list(range(N_CORES)), trace=_trace
    )
    kernel.last_results = res
    out = np.empty((B, S, DIM), dtype=np.float32)
    for g in range(DP):
        acc = res.results[g * TP]["out"].astype(np.float32)
        for r in range(1, TP):
            acc = acc + res.results[g * TP + r]["out"].astype(np.float32)
        # [128, qc, oc, j] -> [oc*128, qc*512]
        outT = acc.transpose(2, 0, 1, 3).reshape(DIM, S)
        out[g] = outT.T
    return out


# revision 16
# speedup vs baseline: 1.4784x; 1.0172x over previous
"""Multi-head causal self-attention with RoPE on 8 Trainium2 NeuronCores.

Sharding: DP(2) x TP(4). Cores [4g, 4g+4) own batch g; within a group,
core r owns heads [4r, 4r+4) (rows [r*512,(r+1)*512) of Wq/Wk/Wv and the
matching columns of Wo). The host sums the 4 partial output projections
per batch (replaces the TP all-reduce); partial sums travel as fp16.

Performance notes (measured on TRN2):
  - PE matmul issue rate is N cycles @2.4GHz regardless of operand dtype
    (fp16 == bf16 == weight-reuse); the kernel is PE-streaming-bound, so
    everything else is organized to keep the PE FIFO dense.
  - dma_start issue on the Sync engine costs ~2.7ns per descriptor line;
    all DRAM tensors are pre-tiled on the host so every transfer is 128
    long per-partition-contiguous descriptors (~0.35us issue each).
  - Scalar activations pay a ~370-cycle access-latency adder, so exps are
    batched two k-chunks per call (st pairs span 2 PSUM banks; engines
    other than the PE may read across banks).
  - Softmax denominators: full (sub-diagonal) k-chunks are summed in
    quads on the DVE (fp16, 2x mode) with one ones-matmul per quad;
    diagonal chunks get individual column-trimmed ones-matmuls.
  - Causality: diagonal-band score chunks are column-trimmed to
    N = 512-128*di and masked multiplicatively (after exp) with a single
    [128,128] triangular 0/1 mask.
"""

import sys

import numpy as np

B, S, DIM = 2, 2048, 2048
NUM_HEADS = 16
HD = 128
N_CORES = 8
DP = 2                       # data-parallel groups (one batch each)
TP = N_CORES // DP           # tensor-parallel ranks per group
HPC = NUM_HEADS // TP        # heads per core (4)
DLOC = HPC * HD              # per-core slice of the model dim (512)
ROPE_BASE = 10000.0
SC = 512                     # s-chunk for projections / attention q-chunk

_PROGRAM_CACHE = {}


def _rope_tables_T(seq_len, head_dim):
    # match reference float32 arithmetic: inv_freq over even indices,
    # emb = cat(freqs, freqs); returned transposed [head_dim, seq_len]
    inv_freq = (
        1.0
        / (np.float32(ROPE_BASE)
           ** (np.arange(0, head_dim, 2, dtype=np.float32) / np.float32(head_dim)))
    ).astype(np.float32)
    t = np.arange(seq_len, dtype=np.float32)
    freqs = np.outer(t, inv_freq).astype(np.float32)      # [S, D/2]
    emb = np.concatenate([freqs, freqs], axis=-1)         # [S, D]
    return (
        np.ascontiguousarray(np.cos(emb).astype(np.float16).T),
        np.ascontiguousarray(np.sin(emb).astype(np.float16).T),
    )


def _rot_matrix_T(head_dim):
    # rotated = cat(-x[1::2], x[::2]) = R @ x; return R.T [D, D]
    d2 = head_dim // 2
    R = np.zeros((head_dim, head_dim), dtype=np.float16)
    for dp in range(d2):
        R[dp, 2 * dp + 1] = -1.0
    for dp in range(d2, head_dim):
        R[dp, 2 * (dp - d2)] = 1.0
    return np.ascontiguousarray(R.T)


def _tri01():
    # tri01[kk, qq] = 1 if kk <= qq else 0 (multiplicative causal mask for
    # the [128,128] diagonal block of every diagonal k-chunk)
    kk = np.arange(128)[:, None]
    qq = np.arange(128)[None, :]
    return np.ascontiguousarray((kk <= qq).astype(np.float16))


def build_program(s=S, dim=DIM):
    """Per-core SPMD Bass program (identical on every core)."""
    if "/opt/trn_rl_repo" not in sys.path:
        sys.path.insert(0, "/opt/trn_rl_repo")
    import concourse.bacc as bacc
    import concourse.mybir as mybir
    import concourse.tile as tile

    f32 = mybir.dt.float32
    f16 = mybir.dt.float16
    EXP = mybir.ActivationFunctionType.Exp

    n_din = dim // 128          # contraction chunks for projections (16)
    n_sc = s // SC              # s-chunks (4)
    n_oc = dim // 128           # output-projection row chunks (16)
    scale = float(HD) ** -0.5

    nc = bacc.Bacc("TRN2", target_bir_lowering=False, debug=False)

    # all DRAM tensors pre-tiled on the host: partition dim first, then
    # per-partition-contiguous free dims, so DMAs are 128 fat descriptors
    x_d = nc.dram_tensor("x", [128, n_sc, n_din, SC], f16, kind="ExternalInput")
    wq_d = nc.dram_tensor("wq", [128, HPC, n_din, HD], f16, kind="ExternalInput")
    wk_d = nc.dram_tensor("wk", [128, HPC, n_din, HD], f16, kind="ExternalInput")
    wv_d = nc.dram_tensor("wv", [128, n_din, DLOC], f16, kind="ExternalInput")
    wo_d = nc.dram_tensor("wo", [128, HPC, dim], f16, kind="ExternalInput")
    cosT_d = nc.dram_tensor("cosT", [HD, s], f16, kind="ExternalInput")
    sinT_d = nc.dram_tensor("sinT", [HD, s], f16, kind="ExternalInput")
    rT_d = nc.dram_tensor("rT", [HD, HD], f16, kind="ExternalInput")
    ones_d = nc.dram_tensor("ones", [HD, HD], f16, kind="ExternalInput")
    tri_d = nc.dram_tensor("tri", [HD, HD], f16, kind="ExternalInput")
    out_d = nc.dram_tensor("out", [128, n_sc, n_oc, SC], f16, kind="ExternalOutput")

    with tile.TileContext(nc) as tc:
        with tc.tile_pool(name="persist", bufs=1) as persist:
            qT = persist.tile([128, HPC, s], f16)   # roped q, [d, h, s]
            kT = persist.tile([128, HPC, s], f16)
            vS = persist.tile([128, s // 128, DLOC], f16)  # [k, chunk, d]
            uT = persist.tile([128, HPC, s], f16)   # attention out, [d, h, s]

            # ---------------- phase 1: qkv projections + RoPE ----------------
            with (
                tc.tile_pool(name="p1x", bufs=3) as p1x,
                tc.tile_pool(name="p1w", bufs=1) as p1w,
                tc.tile_pool(name="p1t", bufs=2) as p1t,
                tc.tile_pool(name="ps1", bufs=2, space="PSUM") as ps1,
            ):
                # startup criticality order: chain (q,h=0) is paced by xt0's
                # c-groups and wq's h=0 slice; later pieces arrive while
                # earlier chains compute
                xts = [
                    p1x.tile([128, n_din, SC], f16, tag="xt", name=f"xt{si}")
                    for si in range(3)
                ]
                wq_s = p1w.tile([128, HPC, n_din, HD], f16)
                wk_s = p1w.tile([128, HPC, n_din, HD], f16)
                wv_s = p1w.tile([128, n_din, DLOC], f16)
                rTs = persist.tile([HD, HD], f16)
                cosT = persist.tile([HD, s], f16)
                sinT = persist.tile([HD, s], f16)
                gq = n_din // 4
                nc.sync.dma_start(out=xts[0][:, :gq, :], in_=x_d[:, 0, :gq, :])
                nc.sync.dma_start(out=wq_s[:, 0, :, :], in_=wq_d[:, 0, :, :])
                for g0 in range(gq, n_din, gq):
                    sl4 = slice(g0, g0 + gq)
                    nc.sync.dma_start(out=xts[0][:, sl4, :], in_=x_d[:, 0, sl4, :])
                for h in range(1, HPC):
                    nc.sync.dma_start(out=wq_s[:, h, :, :], in_=wq_d[:, h, :, :])
                nc.sync.dma_start(out=rTs, in_=rT_d[:])
                nc.sync.dma_start(out=cosT, in_=cosT_d[:])
                nc.sync.dma_start(out=sinT, in_=sinT_d[:])
                for h in range(HPC):
                    nc.sync.dma_start(out=wk_s[:, h, :, :], in_=wk_d[:, h, :, :])
                nh = n_din // 2
                nc.sync.dma_start(out=xts[1][:, :nh, :], in_=x_d[:, 1, :nh, :])
                nc.sync.dma_start(out=xts[1][:, nh:, :], in_=x_d[:, 1, nh:, :])
                nc.sync.dma_start(out=wv_s[:, :nh, :], in_=wv_d[:, :nh, :])
                nc.sync.dma_start(out=wv_s[:, nh:, :], in_=wv_d[:, nh:, :])
                nc.sync.dma_start(out=xts[2][:, :nh, :], in_=x_d[:, 2, :nh, :])
                nc.sync.dma_start(out=xts[2][:, nh:, :], in_=x_d[:, 2, nh:, :])
                ones = persist.tile([HD, HD], f16)
                nc.sync.dma_start(out=ones, in_=ones_d[:])
                tri01 = persist.tile([HD, HD], f16)
                nc.sync.dma_start(out=tri01, in_=tri_d[:])
                woT_s = persist.tile([128, HPC, dim], f16)
                nc.sync.dma_start(out=woT_s, in_=wo_d[:])

                def finish_rope(raw, store, h, s0):
                    # rot matmul emitted one chain late so the PE never waits
                    # on the scalar-engine raw copy
                    rot = ps1.tile([128, SC], f32, tag="rot")
                    nc.tensor.matmul(rot, lhsT=rTs, rhs=raw, start=True, stop=True)
                    t1 = p1t.tile([128, SC], f16, tag="t1")
                    nc.vector.tensor_mul(t1, raw, cosT[:, s0 : s0 + SC])
                    t2 = p1t.tile([128, SC], f16, tag="t2")
                    nc.vector.tensor_mul(t2, rot, sinT[:, s0 : s0 + SC])
                    nc.gpsimd.tensor_add(store[:, h, s0 : s0 + SC], t1, t2)

                pending = None
                for si in range(n_sc):
                    s0 = si * SC
                    if si < 3:
                        xt = xts[si]
                    else:
                        xt = p1x.tile([128, n_din, SC], f16, tag="xt", name="xt3")
                        nc.sync.dma_start(out=xt, in_=x_d[:, si, :, :])

                    for w_s, store in ((wq_s, qT), (wk_s, kT)):
                        for h in range(HPC):
                            acc = ps1.tile([128, SC], f32, tag="acc")
                            for c in range(n_din):
                                nc.tensor.matmul(
                                    acc,
                                    lhsT=w_s[:, h, c, :],
                                    rhs=xt[:, c, :],
                                    start=(c == 0),
                                    stop=(c == n_din - 1),
                                )
                            raw = p1t.tile([128, SC], f16, tag="raw", bufs=3)
                            nc.scalar.copy(raw, acc)
                            if pending is not None:
                                finish_rope(*pending)
                            pending = (raw, store, h, s0)

                    for sp in range(SC // 256):   # v chains in psum pairs
                        vacc2 = ps1.tile([128, 2, SC], f32, tag="vacc")
                        for j in range(2):
                            sub = sp * 2 + j
                            for c in range(n_din):
                                nc.tensor.matmul(
                                    vacc2[:, j, :],
                                    lhsT=xt[:, c, sub * 128 : (sub + 1) * 128],
                                    rhs=wv_s[:, c, :],
                                    start=(c == 0),
                                    stop=(c == n_din - 1),
                                )
                        if pending is not None:
                            finish_rope(*pending)
                            pending = None
                        vdst = vS[:, si * 4 + sp * 2 : si * 4 + sp * 2 + 2, :]
                        if sp == 0:
                            nc.scalar.copy(vdst, vacc2)
                        else:
                            nc.vector.tensor_copy(vdst, vacc2)

            # ------------- phase 2+3: attention + output projection -------------
            with (
                tc.tile_pool(name="p2", bufs=8) as p2,
                tc.tile_pool(name="p2l", bufs=2) as p2l,
                tc.tile_pool(name="p2r", bufs=2) as p2r,
                tc.tile_pool(name="p3", bufs=2) as p3,
                tc.tile_pool(name="ps_t", bufs=3, space="PSUM") as ps_t,
                tc.tile_pool(name="ps_o", bufs=2, space="PSUM") as ps_o,
            ):
                def phase3_og(qc, og):
                    # one 4-row-chunk group of the output projection for
                    # q-chunk qc: 16 PE matmuls + 2 psum-pair copies + 1 DMA.
                    # Interleaved into the next q-chunk's attention chains as
                    # PE filler while the first exps complete.
                    q0 = qc * SC
                    ot4 = p3.tile([128, 4, SC], f16, tag="ot")
                    for op_ in range(2):
                        pos2 = ps_t.tile([128, 2, SC], f32, tag="st", name="pos")
                        for j in range(2):
                            oc = og * 4 + op_ * 2 + j
                            for h in range(HPC):
                                nc.tensor.matmul(
                                    pos2[:, j, :],
                                    lhsT=woT_s[:, h, oc * 128 : (oc + 1) * 128],
                                    rhs=uT[:, h, q0 : q0 + SC],
                                    start=(h == 0),
                                    stop=(h == HPC - 1),
                                )
                        dst = ot4[:, op_ * 2 : op_ * 2 + 2, :]
                        if op_ == 0:
                            nc.scalar.copy(dst, pos2)
                        else:
                            nc.vector.tensor_copy(dst, pos2)
                    nc.sync.dma_start(
                        out=out_d[:, qc, og * 4 : (og + 1) * 4, :], in_=ot4
                    )

                # attention q-chunks processed in rotated order so every
                # chain (including the short qc=0 ones) carries an output-
                # projection filler group from the previously finished chunk.
                # Each chain's epilogue (denominator matmul + reciprocal +
                # normalize) is lagged into the next chain so the PE never
                # waits on the DVE accumulation at head boundaries.
                qc_order = list(range(1, n_sc)) + [0]
                epi_pend = []

                def flush_epi():
                    while epi_pend:
                        outp, lall, h, q0 = epi_pend.pop(0)
                        lrep = ps_t.tile(
                            [128, 2, SC], f32, tag="st", name="lrep"
                        )
                        nc.tensor.matmul(
                            lrep[:, 0, :], lhsT=ones, rhs=lall,
                            start=True, stop=True,
                        )
                        rec = p2r.tile([128, SC], f32, tag="rec")
                        nc.vector.reciprocal_approx_fast(rec, lrep[:, 0, :])
                        nc.vector.tensor_mul(uT[:, h, q0 : q0 + SC], outp, rec)

                for oi, qc in enumerate(qc_order):
                    q0 = qc * SC
                    nfull = 4 * qc          # full (sub-diagonal) k-chunks
                    nkc = nfull + 4
                    prev_qc = qc_order[oi - 1] if oi > 0 else None
                    for h in range(HPC):
                        outp = ps_o.tile([128, SC], f32, tag="o")
                        lall = p2l.tile([128, SC], f16, tag="lp")
                        lst = [False]       # lall initialized?
                        pend_av = []        # (kc, pt AP, co) awaiting AV
                        filler = [prev_qc] if prev_qc is not None else []

                        def mid_chain():
                            # previous chain's epilogue, then the output-
                            # projection filler group, as PE work while this
                            # chain's first exps complete
                            flush_epi()
                            if filler:
                                phase3_og(filler.pop(), h)

                        def lacc(ap, co):
                            # fp16 DVE accumulation of the softmax denominator
                            if not lst[0]:
                                nc.vector.tensor_copy(lall, ap)
                                lst[0] = True
                            else:
                                nc.vector.tensor_add(
                                    lall[:, co:], lall[:, co:], ap
                                )

                        def flush_av(upto):
                            # AV matmuls lag the score/exp stream to keep exp
                            # latency off the PE critical path
                            while len(pend_av) > upto:
                                kc, pt_ap, co = pend_av.pop(0)
                                nc.tensor.matmul(
                                    outp[:, co:],
                                    lhsT=vS[:, kc, h * HD : (h + 1) * HD],
                                    rhs=pt_ap,
                                    start=(kc == 0),
                                    stop=(kc == nkc - 1),
                                )

                        # --- full chunks, exp'd in pairs ---
                        for pr in range(nfull // 2):
                            st2 = ps_t.tile([128, 2, SC], f32, tag="st")
                            pt2 = p2.tile([128, 2, SC], f16, tag="pt")
                            for j in range(2):
                                kc = pr * 2 + j
                                nc.tensor.matmul(
                                    st2[:, j, :],
                                    lhsT=kT[:, h, kc * 128 : (kc + 1) * 128],
                                    rhs=qT[:, h, q0 : q0 + SC],
                                    start=True,
                                    stop=True,
                                )
                            nc.scalar.activation(pt2, st2, EXP, scale=scale)
                            if pr == 0:
                                mid_chain()
                            if lst[0]:
                                nc.vector.tensor_add(lall, lall, pt2[:, 0, :])
                            else:
                                nc.vector.tensor_add(
                                    lall, pt2[:, 0, :], pt2[:, 1, :]
                                )
                                lst[0] = True
                            if pr > 0:
                                nc.vector.tensor_add(lall, lall, pt2[:, 1, :])
                            pend_av.append((pr * 2, pt2[:, 0, :], 0))
                            pend_av.append((pr * 2 + 1, pt2[:, 1, :], 0))
                            flush_av(3)

                        # --- diagonal chunks, trimmed + masked ---
                        for di in range(4):
                            kc = nfull + di
                            co = 128 * di
                            st2 = ps_t.tile([128, 2, SC], f32, tag="st")
                            pt2 = p2.tile([128, 2, SC], f16, tag="pt")
                            nc.tensor.matmul(
                                st2[:, 0, co:],
                                lhsT=kT[:, h, kc * 128 : (kc + 1) * 128],
                                rhs=qT[:, h, q0 + co : q0 + SC],
                                start=True,
                                stop=True,
                            )
                            nc.scalar.activation(
                                pt2[:, 0, co:], st2[:, 0, co:], EXP, scale=scale
                            )
                            if di == 0 and nfull == 0:
                                mid_chain()
                            nc.vector.tensor_mul(
                                pt2[:, 0, co : co + 128],
                                pt2[:, 0, co : co + 128],
                                tri01,
                            )
                            lacc(pt2[:, 0, co:], co)
                            pend_av.append((kc, pt2[:, 0, co:], co))
                            flush_av(3)
                        flush_av(0)
                        epi_pend.append((outp, lall, h, q0))
                flush_epi()
                for og in range(n_oc // 4):
                    phase3_og(qc_order[-1], og)

    nc.compile()
    return nc


def make_in_maps(x, Wq, Wk, Wv, Wo):
    cosT, sinT = _rope_tables_T(S, HD)
    rT = _rot_matrix_T(HD)
    ones = np.ones((HD, HD), dtype=np.float16)
    tri = _tri01()
    n_din, n_sc = DIM // 128, S // SC
    xts = []
    for g in range(DP):
        xT = x[g].T.astype(np.float16)                      # [din, s]
        xts.append(np.ascontiguousarray(
            xT.reshape(n_din, 128, n_sc, SC).transpose(1, 2, 0, 3)
        ))                                                  # [128, si, c, j]
    in_maps = []
    for c in range(N_CORES):
        g, r = divmod(c, TP)
        sl = slice(r * DLOC, (r + 1) * DLOC)

        def tile_w_h(W):
            # [p, h, c, d] = W.T[c*128+p, h*128+d]
            wT = W[sl, :].T.astype(np.float16)              # [din, dloc]
            return np.ascontiguousarray(
                wT.reshape(n_din, 128, HPC, HD).transpose(1, 2, 0, 3)
            )

        def tile_w_c(W):
            wT = W[sl, :].T.astype(np.float16)              # [din, dloc]
            return np.ascontiguousarray(
                wT.reshape(n_din, 128, DLOC).transpose(1, 0, 2)
            )

        woT = Wo[:, sl].T.astype(np.float16)                # [dloc, dim]
        wo_t = np.ascontiguousarray(
            woT.reshape(HPC, 128, DIM).transpose(1, 0, 2)
        )
        in_maps.append(
            {
                "x": xts[g],
                "wq": tile_w_h(Wq),
                "wk": tile_w_h(Wk),
                "wv": tile_w_c(Wv),
                "wo": wo_t,
                "cosT": cosT,
                "sinT": sinT,
                "rT": rT,
                "ones": ones,
                "tri": tri,
            }
        )
    return in_maps


def kernel(x, Wq, Wk, Wv, Wo, _trace=False):
    """Full-input / full-output entry point. Shards over 8 cores internally."""
    if "/opt/trn_rl_repo" not in sys.path:
        sys.path.insert(0, "/opt/trn_rl_repo")
    from concourse.bass_utils import run_bass_kernel_spmd

    x = np.asarray(x, dtype=np.float32)
    Wq, Wk, Wv, Wo = (np.asarray(w, dtype=np.float32) for w in (Wq, Wk, Wv, Wo))

    key = (B, S, DIM)
    if key not in _PROGRAM_CACHE:
        _PROGRAM_CACHE[key] = build_program(S, DIM)
    nc = _PROGRAM_CACHE[key]

    in_maps = make_in_maps(x, Wq, Wk, Wv, Wo)
    res = run_bass_kernel_spmd(
        nc, in_maps, core_ids=list(range(N_CORES)), trace=_trace
    )
    kernel.last_results = res
    out = np.empty((B, S, DIM), dtype=np.float32)
    for g in range(DP):
        acc = res.results[g * TP]["out"].astype(np.float32)
        for r in range(1, TP):
            acc = acc + res.results[g * TP + r]["out"].astype(np.float32)
        # [128, qc, oc, j] -> [oc*128, qc*512]
        outT = acc.transpose(2, 0, 1, 3).reshape(DIM, S)
        out[g] = outT.T
    return out


# revision 17
# speedup vs baseline: 1.4814x; 1.0020x over previous
"""Multi-head causal self-attention with RoPE on 8 Trainium2 NeuronCores.

Sharding: DP(2) x TP(4). Cores [4g, 4g+4) own batch g; within a group,
core r owns heads [4r, 4r+4) (rows [r*512,(r+1)*512) of Wq/Wk/Wv and the
matching columns of Wo). The host sums the 4 partial output projections
per batch (replaces the TP all-reduce); partial sums travel as fp16.

Performance notes (measured on TRN2):
  - PE matmul issue rate is N cycles @2.4GHz regardless of operand dtype
    (fp16 == bf16 == weight-reuse); the kernel is PE-streaming-bound, so
    everything else is organized to keep the PE FIFO dense.
  - dma_start issue on the Sync engine costs ~2.7ns per descriptor line;
    all DRAM tensors are pre-tiled on the host so every transfer is 128
    long per-partition-contiguous descriptors (~0.35us issue each).
  - Scalar activations pay a ~370-cycle access-latency adder, so exps are
    batched two k-chunks per call (st pairs span 2 PSUM banks; engines
    other than the PE may read across banks).
  - Softmax denominators: full (sub-diagonal) k-chunks are summed in
    quads on the DVE (fp16, 2x mode) with one ones-matmul per quad;
    diagonal chunks get individual column-trimmed ones-matmuls.
  - Causality: diagonal-band score chunks are column-trimmed to
    N = 512-128*di and masked multiplicatively (after exp) with a single
    [128,128] triangular 0/1 mask.
"""

import sys

import numpy as np

B, S, DIM = 2, 2048, 2048
NUM_HEADS = 16
HD = 128
N_CORES = 8
DP = 2                       # data-parallel groups (one batch each)
TP = N_CORES // DP           # tensor-parallel ranks per group
HPC = NUM_HEADS // TP        # heads per core (4)
DLOC = HPC * HD              # per-core slice of the model dim (512)
ROPE_BASE = 10000.0
SC = 512                     # s-chunk for projections / attention q-chunk

_PROGRAM_CACHE = {}


def _rope_tables_T(seq_len, head_dim):
    # match reference float32 arithmetic: inv_freq over even indices,
    # emb = cat(freqs, freqs); returned transposed [head_dim, seq_len]
    inv_freq = (
        1.0
        / (np.float32(ROPE_BASE)
           ** (np.arange(0, head_dim, 2, dtype=np.float32) / np.float32(head_dim)))
    ).astype(np.float32)
    t = np.arange(seq_len, dtype=np.float32)
    freqs = np.outer(t, inv_freq).astype(np.float32)      # [S, D/2]
    emb = np.concatenate([freqs, freqs], axis=-1)         # [S, D]
    return (
        np.ascontiguousarray(np.cos(emb).astype(np.float16).T),
        np.ascontiguousarray(np.sin(emb).astype(np.float16).T),
    )


def _rot_matrix_T(head_dim):
    # rotated = cat(-x[1::2], x[::2]) = R @ x; return R.T [D, D]
    d2 = head_dim // 2
    R = np.zeros((head_dim, head_dim), dtype=np.float16)
    for dp in range(d2):
        R[dp, 2 * dp + 1] = -1.0
    for dp in range(d2, head_dim):
        R[dp, 2 * (dp - d2)] = 1.0
    return np.ascontiguousarray(R.T)


def _tri01():
    # tri01[kk, qq] = 1 if kk <= qq else 0 (multiplicative causal mask for
    # the [128,128] diagonal block of every diagonal k-chunk)
    kk = np.arange(128)[:, None]
    qq = np.arange(128)[None, :]
    return np.ascontiguousarray((kk <= qq).astype(np.float16))


def build_program(s=S, dim=DIM):
    """Per-core SPMD Bass program (identical on every core)."""
    if "/opt/trn_rl_repo" not in sys.path:
        sys.path.insert(0, "/opt/trn_rl_repo")
    import concourse.bacc as bacc
    import concourse.mybir as mybir
    import concourse.tile as tile

    f32 = mybir.dt.float32
    f16 = mybir.dt.float16
    EXP = mybir.ActivationFunctionType.Exp

    n_din = dim // 128          # contraction chunks for projections (16)
    n_sc = s // SC              # s-chunks (4)
    n_oc = dim // 128           # output-projection row chunks (16)
    scale = float(HD) ** -0.5

    nc = bacc.Bacc("TRN2", target_bir_lowering=False, debug=False)

    # all DRAM tensors pre-tiled on the host: partition dim first, then
    # per-partition-contiguous free dims, so DMAs are 128 fat descriptors
    x_d = nc.dram_tensor("x", [128, n_sc, n_din, SC], f16, kind="ExternalInput")
    wq_d = nc.dram_tensor("wq", [128, HPC, n_din, HD], f16, kind="ExternalInput")
    wk_d = nc.dram_tensor("wk", [128, HPC, n_din, HD], f16, kind="ExternalInput")
    wv_d = nc.dram_tensor("wv", [128, n_din, DLOC], f16, kind="ExternalInput")
    wo_d = nc.dram_tensor("wo", [128, HPC, dim], f16, kind="ExternalInput")
    cosT_d = nc.dram_tensor("cosT", [HD, s], f16, kind="ExternalInput")
    sinT_d = nc.dram_tensor("sinT", [HD, s], f16, kind="ExternalInput")
    rT_d = nc.dram_tensor("rT", [HD, HD], f16, kind="ExternalInput")
    ones_d = nc.dram_tensor("ones", [HD, HD], f16, kind="ExternalInput")
    tri_d = nc.dram_tensor("tri", [HD, HD], f16, kind="ExternalInput")
    out_d = nc.dram_tensor("out", [128, n_sc, n_oc, SC], f16, kind="ExternalOutput")

    with tile.TileContext(nc) as tc:
        with tc.tile_pool(name="persist", bufs=1) as persist:
            qT = persist.tile([128, HPC, s], f16)   # roped q, [d, h, s]
            kT = persist.tile([128, HPC, s], f16)
            vS = persist.tile([128, s // 128, DLOC], f16)  # [k, chunk, d]
            uT = persist.tile([128, HPC, s], f16)   # attention out, [d, h, s]

            # ---------------- phase 1: qkv projections + RoPE ----------------
            with (
                tc.tile_pool(name="p1x", bufs=3) as p1x,
                tc.tile_pool(name="p1w", bufs=1) as p1w,
                tc.tile_pool(name="p1t", bufs=2) as p1t,
                tc.tile_pool(name="ps1", bufs=2, space="PSUM") as ps1,
            ):
                # startup criticality order: chain (q,h=0) is paced by xt0's
                # c-groups and wq's h=0 slice; later pieces arrive while
                # earlier chains compute
                xts = [
                    p1x.tile([128, n_din, SC], f16, tag="xt", name=f"xt{si}")
                    for si in range(3)
                ]
                wq_s = p1w.tile([128, HPC, n_din, HD], f16)
                wk_s = p1w.tile([128, HPC, n_din, HD], f16)
                wv_s = p1w.tile([128, n_din, DLOC], f16)
                rTs = persist.tile([HD, HD], f16)
                cosT = persist.tile([HD, s], f16)
                sinT = persist.tile([HD, s], f16)
                gq = n_din // 4
                nc.sync.dma_start(out=xts[0][:, :gq, :], in_=x_d[:, 0, :gq, :])
                nc.sync.dma_start(out=wq_s[:, 0, :, :], in_=wq_d[:, 0, :, :])
                for g0 in range(gq, n_din, gq):
                    sl4 = slice(g0, g0 + gq)
                    nc.sync.dma_start(out=xts[0][:, sl4, :], in_=x_d[:, 0, sl4, :])
                for h in range(1, HPC):
                    nc.sync.dma_start(out=wq_s[:, h, :, :], in_=wq_d[:, h, :, :])
                nc.sync.dma_start(out=rTs, in_=rT_d[:])
                nc.sync.dma_start(out=cosT, in_=cosT_d[:])
                nc.sync.dma_start(out=sinT, in_=sinT_d[:])
                for h in range(HPC):
                    nc.sync.dma_start(out=wk_s[:, h, :, :], in_=wk_d[:, h, :, :])
                nh = n_din // 2
                nc.sync.dma_start(out=xts[1][:, :nh, :], in_=x_d[:, 1, :nh, :])
                nc.sync.dma_start(out=xts[1][:, nh:, :], in_=x_d[:, 1, nh:, :])
                nc.sync.dma_start(out=wv_s[:, :nh, :], in_=wv_d[:, :nh, :])
                nc.sync.dma_start(out=wv_s[:, nh:, :], in_=wv_d[:, nh:, :])
                nc.sync.dma_start(out=xts[2][:, :nh, :], in_=x_d[:, 2, :nh, :])
                nc.sync.dma_start(out=xts[2][:, nh:, :], in_=x_d[:, 2, nh:, :])
                ones = persist.tile([HD, HD], f16)
                nc.sync.dma_start(out=ones, in_=ones_d[:])
                tri01 = persist.tile([HD, HD], f16)
                nc.sync.dma_start(out=tri01, in_=tri_d[:])
                woT_s = persist.tile([128, HPC, dim], f16)
                nc.sync.dma_start(out=woT_s, in_=wo_d[:])

                def finish_rope(raw, store, h, s0):
                    # rot matmul emitted one chain late so the PE never waits
                    # on the scalar-engine raw copy
                    rot = ps1.tile([128, SC], f32, tag="rot")
                    nc.tensor.matmul(rot, lhsT=rTs, rhs=raw, start=True, stop=True)
                    t1 = p1t.tile([128, SC], f16, tag="t1")
                    nc.vector.tensor_mul(t1, raw, cosT[:, s0 : s0 + SC])
                    t2 = p1t.tile([128, SC], f16, tag="t2")
                    nc.vector.tensor_mul(t2, rot, sinT[:, s0 : s0 + SC])
                    nc.gpsimd.tensor_add(store[:, h, s0 : s0 + SC], t1, t2)

                pending = None
                for si in range(n_sc):
                    s0 = si * SC
                    if si < 3:
                        xt = xts[si]
                    else:
                        xt = p1x.tile([128, n_din, SC], f16, tag="xt", name="xt3")
                        nc.sync.dma_start(out=xt, in_=x_d[:, si, :, :])

                    for w_s, store in ((wq_s, qT), (wk_s, kT)):
                        for h in range(HPC):
                            acc = ps1.tile([128, SC], f32, tag="acc")
                            for c in range(n_din):
                                nc.tensor.matmul(
                                    acc,
                                    lhsT=w_s[:, h, c, :],
                                    rhs=xt[:, c, :],
                                    start=(c == 0),
                                    stop=(c == n_din - 1),
                                )
                            raw = p1t.tile([128, SC], f16, tag="raw", bufs=3)
                            nc.scalar.copy(raw, acc)
                            if pending is not None:
                                finish_rope(*pending)
                            pending = (raw, store, h, s0)

                    for sub in range(SC // 128):   # v chains
                        vacc = ps1.tile([128, SC], f32, tag="vacc")
                        for c in range(n_din):
                            nc.tensor.matmul(
                                vacc,
                                lhsT=xt[:, c, sub * 128 : (sub + 1) * 128],
                                rhs=wv_s[:, c, :],
                                start=(c == 0),
                                stop=(c == n_din - 1),
                            )
                        if pending is not None:
                            finish_rope(*pending)
                            pending = None
                        vdst = vS[:, si * 4 + sub, :]
                        if sub % 2 == 0:
                            nc.scalar.copy(vdst, vacc)
                        else:
                            nc.vector.tensor_copy(vdst, vacc)

            # ------------- phase 2+3: attention + output projection -------------
            with (
                tc.tile_pool(name="p2", bufs=8) as p2,
                tc.tile_pool(name="p2l", bufs=2) as p2l,
                tc.tile_pool(name="p2r", bufs=2) as p2r,
                tc.tile_pool(name="p3", bufs=2) as p3,
                tc.tile_pool(name="ps_t", bufs=3, space="PSUM") as ps_t,
                tc.tile_pool(name="ps_o", bufs=2, space="PSUM") as ps_o,
            ):
                def phase3_og(qc, og):
                    # one 4-row-chunk group of the output projection for
                    # q-chunk qc: 16 PE matmuls + 2 psum-pair copies + 1 DMA.
                    # Interleaved into the next q-chunk's attention chains as
                    # PE filler while the first exps complete.
                    q0 = qc * SC
                    ot4 = p3.tile([128, 4, SC], f16, tag="ot")
                    for op_ in range(2):
                        pos2 = ps_t.tile([128, 2, SC], f32, tag="st", name="pos")
                        for j in range(2):
                            oc = og * 4 + op_ * 2 + j
                            for h in range(HPC):
                                nc.tensor.matmul(
                                    pos2[:, j, :],
                                    lhsT=woT_s[:, h, oc * 128 : (oc + 1) * 128],
                                    rhs=uT[:, h, q0 : q0 + SC],
                                    start=(h == 0),
                                    stop=(h == HPC - 1),
                                )
                        dst = ot4[:, op_ * 2 : op_ * 2 + 2, :]
                        if op_ == 0:
                            nc.scalar.copy(dst, pos2)
                        else:
                            nc.vector.tensor_copy(dst, pos2)
                    nc.sync.dma_start(
                        out=out_d[:, qc, og * 4 : (og + 1) * 4, :], in_=ot4
                    )

                # attention q-chunks processed in rotated order so every
                # chain (including the short qc=0 ones) carries an output-
                # projection filler group from the previously finished chunk.
                # Each chain's epilogue (denominator matmul + reciprocal +
                # normalize) is lagged into the next chain so the PE never
                # waits on the DVE accumulation at head boundaries.
                qc_order = list(range(1, n_sc)) + [0]
                epi_pend = []

                def flush_epi():
                    while epi_pend:
                        outp, lall, h, q0 = epi_pend.pop(0)
                        lrep = ps_t.tile(
                            [128, 2, SC], f32, tag="st", name="lrep"
                        )
                        nc.tensor.matmul(
                            lrep[:, 0, :], lhsT=ones, rhs=lall,
                            start=True, stop=True,
                        )
                        rec = p2r.tile([128, SC], f32, tag="rec")
                        nc.vector.reciprocal_approx_fast(rec, lrep[:, 0, :])
                        nc.vector.tensor_mul(uT[:, h, q0 : q0 + SC], outp, rec)

                for oi, qc in enumerate(qc_order):
                    q0 = qc * SC
                    nfull = 4 * qc          # full (sub-diagonal) k-chunks
                    nkc = nfull + 4
                    prev_qc = qc_order[oi - 1] if oi > 0 else None
                    for h in range(HPC):
                        outp = ps_o.tile([128, SC], f32, tag="o")
                        lall = p2l.tile([128, SC], f16, tag="lp")
                        lst = [False]       # lall initialized?
                        pend_av = []        # (kc, pt AP, co) awaiting AV
                        filler = [prev_qc] if prev_qc is not None else []

                        def mid_chain():
                            # previous chain's epilogue, then the output-
                            # projection filler group, as PE work while this
                            # chain's first exps complete
                            flush_epi()
                            if filler:
                                phase3_og(filler.pop(), h)

                        def lacc(ap, co):
                            # fp16 DVE accumulation of the softmax denominator
                            if not lst[0]:
                                nc.vector.tensor_copy(lall, ap)
                                lst[0] = True
                            else:
                                nc.vector.tensor_add(
                                    lall[:, co:], lall[:, co:], ap
                                )

                        def flush_av(upto):
                            # AV matmuls lag the score/exp stream to keep exp
                            # latency off the PE critical path
                            while len(pend_av) > upto:
                                kc, pt_ap, co = pend_av.pop(0)
                                nc.tensor.matmul(
                                    outp[:, co:],
                                    lhsT=vS[:, kc, h * HD : (h + 1) * HD],
                                    rhs=pt_ap,
                                    start=(kc == 0),
                                    stop=(kc == nkc - 1),
                                )

                        # --- full chunks, exp'd in pairs ---
                        for pr in range(nfull // 2):
                            st2 = ps_t.tile([128, 2, SC], f32, tag="st")
                            pt2 = p2.tile([128, 2, SC], f16, tag="pt")
                            for j in range(2):
                                kc = pr * 2 + j
                                nc.tensor.matmul(
                                    st2[:, j, :],
                                    lhsT=kT[:, h, kc * 128 : (kc + 1) * 128],
                                    rhs=qT[:, h, q0 : q0 + SC],
                                    start=True,
                                    stop=True,
                                )
                            nc.scalar.activation(pt2, st2, EXP, scale=scale)
                            if pr == 0:
                                mid_chain()
                            if lst[0]:
                                nc.vector.tensor_add(lall, lall, pt2[:, 0, :])
                            else:
                                nc.vector.tensor_add(
                                    lall, pt2[:, 0, :], pt2[:, 1, :]
                                )
                                lst[0] = True
                            if pr > 0:
                                nc.vector.tensor_add(lall, lall, pt2[:, 1, :])
                            pend_av.append((pr * 2, pt2[:, 0, :], 0))
                            pend_av.append((pr * 2 + 1, pt2[:, 1, :], 0))
                            flush_av(3)

                        # --- diagonal chunks, trimmed + masked ---
                        for di in range(4):
                            kc = nfull + di
                            co = 128 * di
                            st2 = ps_t.tile([128, 2, SC], f32, tag="st")
                            pt2 = p2.tile([128, 2, SC], f16, tag="pt")
                            nc.tensor.matmul(
                                st2[:, 0, co:],
                                lhsT=kT[:, h, kc * 128 : (kc + 1) * 128],
                                rhs=qT[:, h, q0 + co : q0 + SC],
                                start=True,
                                stop=True,
                            )
                            nc.scalar.activation(
                                pt2[:, 0, co:], st2[:, 0, co:], EXP, scale=scale
                            )
                            if di == 0 and nfull == 0:
                                mid_chain()
                            nc.vector.tensor_mul(
                                pt2[:, 0, co : co + 128],
                                pt2[:, 0, co : co + 128],
                                tri01,
                            )
                            lacc(pt2[:, 0, co:], co)
                            pend_av.append((kc, pt2[:, 0, co:], co))
                            flush_av(3)
                        flush_av(0)
                        epi_pend.append((outp, lall, h, q0))
                flush_epi()
                for og in range(n_oc // 4):
                    phase3_og(qc_order[-1], og)

    nc.compile()
    return nc


def make_in_maps(x, Wq, Wk, Wv, Wo):
    cosT, sinT = _rope_tables_T(S, HD)
    rT = _rot_matrix_T(HD)
    ones = np.ones((HD, HD), dtype=np.float16)
    tri = _tri01()
    n_din, n_sc = DIM // 128, S // SC
    xts = []
    for g in range(DP):
        xT = x[g].T.astype(np.float16)                      # [din, s]
        xts.append(np.ascontiguousarray(
            xT.reshape(n_din, 128, n_sc, SC).transpose(1, 2, 0, 3)
        ))                                                  # [128, si, c, j]
    in_maps = []
    for c in range(N_CORES):
        g, r = divmod(c, TP)
        sl = slice(r * DLOC, (r + 1) * DLOC)

        def tile_w_h(W):
            # [p, h, c, d] = W.T[c*128+p, h*128+d]
            wT = W[sl, :].T.astype(np.float16)              # [din, dloc]
            return np.ascontiguousarray(
                wT.reshape(n_din, 128, HPC, HD).transpose(1, 2, 0, 3)
            )

        def tile_w_c(W):
            wT = W[sl, :].T.astype(np.float16)              # [din, dloc]
            return np.ascontiguousarray(
                wT.reshape(n_din, 128, DLOC).transpose(1, 0, 2)
            )

        woT = Wo[:, sl].T.astype(np.float16)                # [dloc, dim]
        wo_t = np.ascontiguousarray(
            woT.reshape(HPC, 128, DIM).transpose(1, 0, 2)
        )
        in_maps.append(
            {
                "x": xts[g],
                "wq": tile_w_h(Wq),
                "wk": tile_w_h(Wk),
                "wv": tile_w_c(Wv),
                "wo": wo_t,
                "cosT": cosT,
                "sinT": sinT,
                "rT": rT,
                "ones": ones,
                "tri": tri,
            }
        )
    return in_maps


def kernel(x, Wq, Wk, Wv, Wo, _trace=False):
    """Full-input / full-output entry point. Shards over 8 cores internally."""
    if "/opt/trn_rl_repo" not in sys.path:
        sys.path.insert(0, "/opt/trn_rl_repo")
    from concourse.bass_utils import run_bass_kernel_spmd

    x = np.asarray(x, dtype=np.float32)
    Wq, Wk, Wv, Wo = (np.asarray(w, dtype=np.float32) for w in (Wq, Wk, Wv, Wo))

    key = (B, S, DIM)
    if key not in _PROGRAM_CACHE:
        _PROGRAM_CACHE[key] = build_program(S, DIM)
    nc = _PROGRAM_CACHE[key]

    in_maps = make_in_maps(x, Wq, Wk, Wv, Wo)
    res = run_bass_kernel_spmd(
        nc, in_maps, core_ids=list(range(N_CORES)), trace=_trace
    )
    kernel.last_results = res
    out = np.empty((B, S, DIM), dtype=np.float32)
    for g in range(DP):
        acc = res.results[g * TP]["out"].astype(np.float32)
        for r in range(1, TP):
            acc = acc + res.results[g * TP + r]["out"].astype(np.float32)
        # [128, qc, oc, j] -> [oc*128, qc*512]
        outT = acc.transpose(2, 0, 1, 3).reshape(DIM, S)
        out[g] = outT.T
    return out
